# revision 1
# baseline (speedup 1.0000x reference)
"""MLA (multi-head latent attention) forward on 8 TRN2 NeuronCores.

Sharding: core = 4*b + g  (b = batch 0..1, g = head-group 0..3, 4 heads each).
Each core: compress (replicated within batch group) -> decompress its 4 heads
-> causal attention -> partial out-proj.  Host sums the 4 partials per batch.

All matmuls bf16 (fp32 PSUM accumulation).  RMSNorm gains and the RoPE
butterfly (sin==cos bug preserved) are folded into the weights on the host;
the per-token rsqrt factors and the cos table are applied as elementwise
multiplies at PSUM-eviction time.  Softmax skips the max subtraction (logits
are O(10) here) and gets its denominator from an appended ones-column in V.

Activation tiles are split per 512-token window so the Tile scheduler can
overlap compression / decompression / attention / projection; attention
processes two heads per exp (one [128,1024] activation over a 2-bank PSUM
tile) to amortize ACT per-op overhead.
"""

import sys

sys.path.insert(0, "/opt/trn_rl_repo")

import numpy as np
import ml_dtypes

from concourse import bacc, bass, bass_isa, mybir, tile
from concourse.bass_utils import run_bass_kernel_spmd

# problem dims (hardcoded per contract)
B, S, D = 2, 2048, 2048
H = 16
NOPE, ROPE, VD = 64, 32, 64
QR, KVR = 768, 256
EPS = 1e-6
THETA = 10000.0

HG = 4  # heads per core
NCORES = 8
P = 128
W = 512  # token window
NW = S // W  # 4
NT = S // P  # 16
QKD = NOPE + ROPE  # 96

BF = mybir.dt.bfloat16
F32 = mybir.dt.float32
NBF = ml_dtypes.bfloat16
MULT = mybir.AluOpType.mult
AFT = mybir.ActivationFunctionType

LAST_RESULT = None
_CACHE = {}


def _build_nc(loop_n=None, skip_cc=False):
    import contextlib
    nc = bacc.Bacc("TRN2", debug=False)
    with tile.TileContext(nc) as tc:
        with (
            tc.tile_pool(name="dram", bufs=1, space="DRAM") as dram,
            tc.tile_pool(name="wres", bufs=1) as wres,
            tc.tile_pool(name="acts", bufs=1) as acts,
            tc.tile_pool(name="xin", bufs=16) as xin,
            tc.tile_pool(name="sq", bufs=1) as sqp,
            tc.tile_pool(name="pt", bufs=3) as ptp,
            tc.tile_pool(name="stage", bufs=6) as stg,
            tc.tile_pool(name="bc", bufs=3) as bcp,
            tc.tile_pool(name="ps2", bufs=3, space="PSUM") as ps2,
            tc.tile_pool(name="pso", bufs=2, space="PSUM") as pso,
        ):
            # ---------------- DRAM params ----------------
            xTw = dram.tile([D, W], BF, kind="ExternalInput", name="xTw", uniquify=False)
            cropew_d = dram.tile(
                [ROPE, W], BF, kind="ExternalInput", name="cropew", uniquify=False
            )
            wcq = dram.tile([D, QR], BF, kind="ExternalInput", name="wcq", uniquify=False)
            wckvkr = dram.tile(
                [D, KVR + ROPE], BF, kind="ExternalInput", name="wckvkr", uniquify=False
            )
            wq = dram.tile(
                [QR, HG * QKD], BF, kind="ExternalInput", name="wq", uniquify=False
            )
            wkv = dram.tile(
                [KVR, HG * (NOPE + VD)], BF, kind="ExternalInput", name="wkv",
                uniquify=False,
            )
            wproj = dram.tile(
                [HG * VD, D], BF, kind="ExternalInput", name="wproj", uniquify=False
            )
            crope_d = dram.tile(
                [P, S], BF, kind="ExternalInput", name="crope", uniquify=False
            )
            masks_d = dram.tile(
                [4, P, W], BF, kind="ExternalInput", name="masks", uniquify=False
            )
            out_d = dram.tile(
                [S, D], F32, kind="ExternalOutput", name="out", uniquify=False
            )

            # ---------------- resident SBUF ----------------
            wcq_sb = wres.tile([P, D // P, QR], BF, tag="wcq")
            wckvkr_sb = wres.tile([P, D // P, KVR + ROPE], BF, tag="wckvkr")
            wq_sb = wres.tile([P, QR // P, HG * QKD], BF, tag="wq")
            wkv_sb = wres.tile([P, KVR // P, HG * (NOPE + VD)], BF, tag="wkv")
            wproj_sb = wres.tile([P, (HG * VD) // P, D], BF, tag="wproj")
            crope_sb = wres.tile([P, S], BF, tag="crope")
            masks_sb = wres.tile([P, 4, W], BF, tag="masks")
            cb_sb = wres.tile([P, 4], F32, tag="cb")  # [sc_q, b_q, sc_kv, b_kv]

            # loads needed for compression come first
            for c in range(D // P):
                nc.sync.dma_start(out=wcq_sb[:, c, :], in_=wcq[c * P : (c + 1) * P, :])
                nc.sync.dma_start(
                    out=wckvkr_sb[:, c, :], in_=wckvkr[c * P : (c + 1) * P, :]
                )
            nc.sync.dma_start(out=crope_sb[:], in_=crope_d[:])
            cropew_sb = wres.tile([ROPE, W], BF, tag="cropew")
            nc.sync.dma_start(out=cropew_sb[:], in_=cropew_d[:])
            nc.vector.memset(cb_sb[:, 0:1], float(QKD) / QR)
            nc.vector.memset(cb_sb[:, 1:2], float(QKD) * EPS)
            nc.vector.memset(cb_sb[:, 2:3], 1.0 / KVR)
            nc.vector.memset(cb_sb[:, 3:4], EPS)

            # ---------------- per-window activations ----------------
            def wtiles(shape, dt, base, pool=acts):
                return [
                    pool.tile(shape, dt, tag=f"{base}{w}", name=f"{base}{w}")
                    for w in range(NW)
                ]

            cqT_w = wtiles([P, QR // P, W], BF, "cqT")
            ckvT_w = wtiles([P, KVR // P, W], BF, "ckvT")
            krT_w = wtiles([ROPE, W], BF, "krT")
            rqbc_w = wtiles([P, W], F32, "rqbc")
            rkvbc_w = wtiles([P, W], F32, "rkvbc")
            rkvcol_w = wtiles([P, NW], F32, "rkvcol")
            vaug_w = wtiles([P, NW, HG, VD + 1], BF, "vaug")
            oT_w = wtiles([P, 2, W], BF, "oT")
            qT_sb = [
                [
                    acts.tile([QKD, W], BF, tag=f"qT{h}_{w}", name=f"qT{h}_{w}")
                    for w in range(NW)
                ]
                for h in range(HG)
            ]
            kT_sb = [
                [
                    acts.tile([QKD, W], BF, tag=f"kT{h}_{w}", name=f"kT{h}_{w}")
                    for w in range(NW)
                ]
                for h in range(HG)
            ]

            def body():
                # ============ PHASE C: compress OWN 512-token window ============
                # xTw holds only this core's window.  ckv+kr are compressed first
                # and gathered (A) while the cq compression still runs; cq + rq
                # row go in gather B.  Rows are f32 bitcast into the bf16 payload.
                GROUPS = [[0, 1, 2, 3], [4, 5, 6, 7]]
                CKR = KVR + ROPE + 2  # 290: ckv + kr + rkv-row(f32 as 2 bf16 rows)
                CQR = QR + 2  # 770: cq + rq-row
                cc_in = dram.tile(
                    [CKR + CQR, W], BF, kind="Internal", name="cc_in", uniquify=False
                )
                cc_out_a = dram.tile(
                    [NW, CKR, W], BF, kind="Internal", name="cc_out_a", uniquify=False
                )
                cc_out_b = dram.tile(
                    [NW, CQR, W], BF, kind="Internal", name="cc_out_b", uniquify=False
                )

                def gather(in_ap, out_ap):
                    if skip_cc:
                        return
                    nc.gpsimd.collective_compute(
                        "AllGather",
                        mybir.AluOpType.bypass,
                        replica_groups=GROUPS,
                        ins=[in_ap],
                        outs=[out_ap],
                    )

                xts = []
                for c in range(D // P):
                    xt = xin.tile([P, W], BF, tag="xt")
                    nc.sync.dma_start(out=xt[:], in_=xTw[c * P : (c + 1) * P, :])
                    xts.append(xt)
                acc_q = bcp.tile([P, W], F32, tag="sqacc", bufs=2)
                acc_kv = bcp.tile([P, W], F32, tag="sqacc", bufs=2)
                # ---- ckv (2 M-tiles) + kr first ----
                for m in range(KVR // P):
                    psum = ps2.tile([P, W], F32, tag="ps")
                    for c in range(D // P):
                        nc.tensor.matmul(
                            psum[:],
                            wckvkr_sb[:, c, m * P : (m + 1) * P],
                            xts[c][:],
                            start=(c == 0),
                            stop=(c == D // P - 1),
                        )
                    st = stg.tile([P, W], BF, tag="st")
                    nc.scalar.copy(out=st[:], in_=psum[:])
                    nc.sync.dma_start(out=cc_in[m * P : (m + 1) * P, :], in_=st[:])
                    sq = sqp.tile([P, W], BF, tag="sq")
                    nc.scalar.square(out=sq[:], in_=psum[:])
                    if m == 0:
                        nc.vector.tensor_copy(out=acc_kv[:], in_=sq[:])
                    else:
                        nc.vector.tensor_add(out=acc_kv[:], in0=acc_kv[:], in1=sq[:])
                psum = ps2.tile([ROPE, W], F32, tag="ps")
                for c in range(D // P):
                    nc.tensor.matmul(
                        psum[:],
                        wckvkr_sb[:, c, KVR : KVR + ROPE],
                        xts[c][:],
                        start=(c == 0),
                        stop=(c == D // P - 1),
                    )
                st = stg.tile([ROPE, W], BF, tag="st")
                nc.vector.tensor_tensor(out=st[:], in0=psum[:], in1=cropew_sb[:], op=MULT)
                nc.sync.dma_start(out=cc_in[KVR : KVR + ROPE, :], in_=st[:])
                # rkv = rsqrt(mean+eps) row
                t3 = bcp.tile([P, W], F32, tag="tmp2", bufs=2)
                nc.gpsimd.partition_all_reduce(
                    t3[:], acc_kv[:], channels=P, reduce_op=bass_isa.ReduceOp.add
                )
                t4 = bcp.tile([P, W], F32, tag="tmp2", bufs=2)
                nc.scalar.activation(
                    out=t4[0:1, :], in_=t3[0:1, :], func=AFT.Sqrt,
                    bias=cb_sb[0:1, 3:4], scale=cb_sb[0:1, 2:3],
                )
                rowkv = bcp.tile([1, W], F32, tag="row", bufs=2)
                nc.vector.reciprocal(out=rowkv[:], in_=t4[0:1, :])
                nc.sync.dma_start(
                    out=cc_in[KVR + ROPE : KVR + ROPE + 2, :].bitcast(F32), in_=rowkv[:]
                )
                gather(cc_in[0:CKR, :], cc_out_a[:])
                # ---- cq (6 M-tiles) ----
                for m in range(QR // P):
                    psum = ps2.tile([P, W], F32, tag="ps")
                    for c in range(D // P):
                        nc.tensor.matmul(
                            psum[:],
                            wcq_sb[:, c, m * P : (m + 1) * P],
                            xts[c][:],
                            start=(c == 0),
                            stop=(c == D // P - 1),
                        )
                    st = stg.tile([P, W], BF, tag="st")
                    nc.scalar.copy(out=st[:], in_=psum[:])
                    nc.sync.dma_start(
                        out=cc_in[CKR + m * P : CKR + (m + 1) * P, :], in_=st[:]
                    )
                    sq = sqp.tile([P, W], BF, tag="sq")
                    nc.scalar.square(out=sq[:], in_=psum[:])
                    if m == 0:
                        nc.vector.tensor_copy(out=acc_q[:], in_=sq[:])
                    else:
                        nc.vector.tensor_add(out=acc_q[:], in0=acc_q[:], in1=sq[:])
                # rq = rsqrt(96*mean+96*eps) row (folds 1/sqrt(96) score scale)
                t1 = bcp.tile([P, W], F32, tag="tmp2", bufs=2)
                nc.gpsimd.partition_all_reduce(
                    t1[:], acc_q[:], channels=P, reduce_op=bass_isa.ReduceOp.add
                )
                t2 = bcp.tile([P, W], F32, tag="tmp2", bufs=2)
                nc.scalar.activation(
                    out=t2[0:1, :], in_=t1[0:1, :], func=AFT.Sqrt,
                    bias=cb_sb[0:1, 1:2], scale=cb_sb[0:1, 0:1],
                )
                rowq = bcp.tile([1, W], F32, tag="row", bufs=2)
                nc.vector.reciprocal(out=rowq[:], in_=t2[0:1, :])
                nc.sync.dma_start(
                    out=cc_in[CKR + QR : CKR + QR + 2, :].bitcast(F32), in_=rowq[:]
                )
                gather(cc_in[CKR : CKR + CQR, :], cc_out_b[:])

                # ---- fill per-window tiles from the gathered latents ----
                for w in range(NW):
                    for m in range(KVR // P):
                        nc.sync.dma_start(
                            out=ckvT_w[w][:, m, :],
                            in_=cc_out_a[w, m * P : (m + 1) * P, :],
                        )
                    nc.sync.dma_start(
                        out=krT_w[w][:], in_=cc_out_a[w, KVR : KVR + ROPE, :]
                    )
                    rkvrow_t = bcp.tile([1, W], F32, tag="row", bufs=2)
                    nc.sync.dma_start(
                        out=rkvrow_t[:],
                        in_=cc_out_a[w, KVR + ROPE : KVR + ROPE + 2, :].bitcast(F32),
                    )
                    nc.gpsimd.partition_broadcast(rkvbc_w[w][:], rkvrow_t[:])
                    nc.sync.dma_start(
                        out=rkvcol_w[w][:],
                        in_=cc_out_a[w, KVR + ROPE : KVR + ROPE + 2, :]
                        .bitcast(F32)
                        .rearrange("a (c p) -> p (a c)", p=P),
                    )
                    for m in range(QR // P):
                        nc.sync.dma_start(
                            out=cqT_w[w][:, m, :],
                            in_=cc_out_b[w, m * P : (m + 1) * P, :],
                        )
                    rqrow_t = bcp.tile([1, W], F32, tag="row", bufs=2)
                    nc.sync.dma_start(
                        out=rqrow_t[:],
                        in_=cc_out_b[w, QR : QR + 2, :].bitcast(F32),
                    )
                    nc.gpsimd.partition_broadcast(rqbc_w[w][:], rqrow_t[:])

                # decompress/attention weights (scheduled while compression runs)
                for c in range(QR // P):
                    nc.sync.dma_start(out=wq_sb[:, c, :], in_=wq[c * P : (c + 1) * P, :])
                for c in range(KVR // P):
                    nc.sync.dma_start(out=wkv_sb[:, c, :], in_=wkv[c * P : (c + 1) * P, :])
                for t in range(4):
                    nc.sync.dma_start(out=masks_sb[:, t, :], in_=masks_d[t])

                # ============ PHASE D: decompress (per window, kv first) ============
                for w in range(NW):
                    ws = slice(w * W, (w + 1) * W)
                    # k_nope in head pairs
                    for i in range(HG // 2):
                        psum = ps2.tile([P, W], F32, tag="ps")
                        for r in range(KVR // P):
                            nc.tensor.matmul(
                                psum[:],
                                wkv_sb[:, r, i * P : (i + 1) * P],
                                ckvT_w[w][:, r, :],
                                start=(r == 0),
                                stop=(r == KVR // P - 1),
                            )
                        for j in range(2):
                            h = 2 * i + j
                            nc.vector.tensor_tensor(
                                out=kT_sb[h][w][0:NOPE, :],
                                in0=psum[NOPE * j : NOPE * (j + 1), :],
                                in1=rkvbc_w[w][0:NOPE, :],
                                op=MULT,
                            )
                    for h in range(HG):
                        nc.vector.tensor_copy(
                            out=kT_sb[h][w][NOPE:QKD, :], in_=krT_w[w][:]
                        )
                    # v (token-major) + ones column
                    nc.vector.memset(vaug_w[w][:, :, :, VD : VD + 1], 1.0)
                    for cc in range(NW):
                        psum = ps2.tile([P, HG * VD], F32, tag="ps")
                        for r in range(KVR // P):
                            nc.tensor.matmul(
                                psum[:],
                                ckvT_w[w][:, r, cc * P : (cc + 1) * P],
                                wkv_sb[:, r, HG * NOPE : HG * (NOPE + VD)],
                                start=(r == 0),
                                stop=(r == KVR // P - 1),
                            )
                        nc.scalar.activation(
                            out=vaug_w[w][:, cc, :, 0:VD],
                            in_=psum[:].rearrange("p (h d) -> p h d", h=HG),
                            func=AFT.Copy,
                            scale=rkvcol_w[w][:, cc : cc + 1],
                        )
                for w in range(NW):
                    ws = slice(w * W, (w + 1) * W)
                    for h in range(HG):
                        psum = ps2.tile([QKD, W], F32, tag="ps")
                        for r in range(QR // P):
                            nc.tensor.matmul(
                                psum[:],
                                wq_sb[:, r, h * QKD : (h + 1) * QKD],
                                cqT_w[w][:, r, :],
                                start=(r == 0),
                                stop=(r == QR // P - 1),
                            )
                        nc.vector.tensor_tensor(
                            out=qT_sb[h][w][0:NOPE, :],
                            in0=psum[0:NOPE, :],
                            in1=rqbc_w[w][0:NOPE, :],
                            op=MULT,
                        )
                        nc.vector.tensor_tensor(
                            out=qT_sb[h][w][NOPE:QKD, :],
                            in0=psum[NOPE:QKD, :],
                            in1=crope_sb[ROPE * h : ROPE * (h + 1), ws],
                            op=MULT,
                        )
                        nc.vector.tensor_tensor(
                            out=qT_sb[h][w][NOPE:QKD, :],
                            in0=qT_sb[h][w][NOPE:QKD, :],
                            in1=rqbc_w[w][NOPE:QKD, :],
                            op=MULT,
                        )

                # ============ PHASE A: attention (w outer, head pairs) ============
                for w in range(NW):
                    nkc = 4 * w + 4
                    for hp in range(HG // 2):
                        h0, h1 = 2 * hp, 2 * hp + 1
                        op0 = pso.tile([VD + 1, W], F32, tag="ot")
                        op1 = pso.tile([VD + 1, W], F32, tag="ot")
                        for kc in range(nkc):
                            wk, ck = divmod(kc, NW)
                            cs = slice(ck * P, (ck + 1) * P)
                            sp = ps2.tile([P, 2 * W], F32, tag="ps")
                            nc.tensor.matmul(
                                sp[:, 0:W],
                                kT_sb[h0][wk][:, cs],
                                qT_sb[h0][w][:],
                                start=True,
                                stop=True,
                            )
                            nc.tensor.matmul(
                                sp[:, W : 2 * W],
                                kT_sb[h1][wk][:, cs],
                                qT_sb[h1][w][:],
                                start=True,
                                stop=True,
                            )
                            pt = ptp.tile([P, 2 * W], BF, tag="pt")
                            nc.scalar.activation(out=pt[:], in_=sp[:], func=AFT.Exp)
                            t = kc - 4 * w
                            if t >= 0:
                                nc.vector.tensor_mul(
                                    out=pt[:, 0:W], in0=pt[:, 0:W], in1=masks_sb[:, t, :]
                                )
                                nc.vector.tensor_mul(
                                    out=pt[:, W : 2 * W],
                                    in0=pt[:, W : 2 * W],
                                    in1=masks_sb[:, t, :],
                                )
                            nc.tensor.matmul(
                                op0[:],
                                vaug_w[wk][:, ck, h0, :],
                                pt[:, 0:W],
                                start=(kc == 0),
                                stop=(kc == nkc - 1),
                            )
                            nc.tensor.matmul(
                                op1[:],
                                vaug_w[wk][:, ck, h1, :],
                                pt[:, W : 2 * W],
                                start=(kc == 0),
                                stop=(kc == nkc - 1),
                            )
                        for j, op in ((0, op0), (1, op1)):
                            h = 2 * hp + j
                            rec = bcp.tile([1, W], F32, tag="row", bufs=2)
                            nc.vector.reciprocal(out=rec[:], in_=op[VD : VD + 1, :])
                            recb = bcp.tile([P, W], F32, tag="recb", bufs=2)
                            nc.gpsimd.partition_broadcast(recb[:], rec[:])
                            nc.vector.tensor_tensor(
                                out=oT_w[w][NOPE * (h % 2) : NOPE * (h % 2) + VD, h // 2, :],
                                in0=op[0:VD, :],
                                in1=recb[0:VD, :],
                                op=MULT,
                            )

                # proj weights late
                for c in range((HG * VD) // P):
                    nc.sync.dma_start(
                        out=wproj_sb[:, c, :], in_=wproj[c * P : (c + 1) * P, :]
                    )

                # ============ PHASE P: projection ============
                for w in range(NW):  # token window
                    for tt in range(NW):  # token chunk within window
                        t = NW * w + tt
                        for wc in range(NW):  # output column window
                            wcs = slice(wc * W, (wc + 1) * W)
                            psum = ps2.tile([P, W], F32, tag="ps")
                            for i in range(2):
                                nc.tensor.matmul(
                                    psum[:],
                                    oT_w[w][:, i, tt * P : (tt + 1) * P],
                                    wproj_sb[:, i, wcs],
                                    start=(i == 0),
                                    stop=(i == 1),
                                )
                            st = stg.tile([P, W], F32, tag="st")
                            if wc % 2 == 0:
                                nc.vector.tensor_copy(out=st[:], in_=psum[:])
                            else:
                                nc.scalar.copy(out=st[:], in_=psum[:])
                            nc.sync.dma_start(
                                out=out_d[t * P : (t + 1) * P, wcs], in_=st[:]
                            )


            if loop_n:
                with tc.For_i(0, loop_n, 1):
                    body()
            else:
                body()

    nc.compile()
    return nc


def _rope_fold():
    """32x32 butterfly for RoPE with the reference's sin==cos bug."""
    Bm = np.zeros((ROPE, ROPE), np.float32)
    for j in range(ROPE // 2):
        Bm[2 * j, 2 * j] = 1.0
        Bm[2 * j, 2 * j + 1] = -1.0
        Bm[2 * j + 1, 2 * j] = 1.0
        Bm[2 * j + 1, 2 * j + 1] = 1.0
    return Bm


def _host_tables():
    freqs = 1.0 / (THETA ** (np.arange(0, ROPE, 2, dtype=np.float32) / ROPE))
    ang = np.outer(np.arange(S, dtype=np.float32), freqs)  # [S, 16]
    cos = np.cos(ang)  # [S, 16]
    crope32 = np.repeat(cos, 2, axis=1).T.copy()  # [32, S]
    crope = np.tile(crope32, (4, 1)).astype(NBF)  # [128, S]
    masks = np.zeros((4, P, W), np.float32)
    for t in range(4):
        for i_ in range(P):
            j0 = P * t + i_
            if j0 < W:
                masks[t, i_, j0:] = 1.0
    return crope, masks.astype(NBF)


def kernel(**inputs):
    global LAST_RESULT
    x = np.asarray(inputs["x"], np.float32)
    w_cq = np.asarray(inputs["w_cq"], np.float32)
    w_q_nope = np.asarray(inputs["w_q_nope"], np.float32)
    w_q_rope = np.asarray(inputs["w_q_rope"], np.float32)
    q_g = np.asarray(inputs["q_g"], np.float32)
    w_ckv = np.asarray(inputs["w_ckv"], np.float32)
    w_k_nope = np.asarray(inputs["w_k_nope"], np.float32)
    w_v = np.asarray(inputs["w_v"], np.float32)
    kv_g = np.asarray(inputs["kv_g"], np.float32)
    w_k_rope = np.asarray(inputs["w_k_rope"], np.float32)
    w_proj = np.asarray(inputs["w_proj"], np.float32)

    Bm = _rope_fold()
    crope, masks = _host_tables()

    wqn = w_q_nope * q_g[:, None]  # [QR, H*64]
    wqr = w_q_rope * q_g[:, None]  # [QR, H*32]
    wkn = w_k_nope * kv_g[:, None]  # [KVR, H*64]
    wv = w_v * kv_g[:, None]  # [KVR, H*64]
    wkr = (w_k_rope @ Bm.T) / H  # [D, 32]
    wckvkr = np.concatenate([w_ckv, wkr], axis=1)  # [D, 288]

    if "nc" not in _CACHE:
        _CACHE["nc"] = _build_nc()
    nc = _CACHE["nc"]

    in_maps = []
    for core in range(NCORES):
        b, g = divmod(core, NCORES // B)
        heads = range(HG * g, HG * (g + 1))
        wq_cols = []
        for h in heads:
            wq_cols.append(wqn[:, h * NOPE : (h + 1) * NOPE])
            wq_cols.append(wqr[:, h * ROPE : (h + 1) * ROPE] @ Bm.T)
        wq_core = np.concatenate(wq_cols, axis=1)  # [QR, 384]
        wkv_core = np.concatenate(
            [wkn[:, h * NOPE : (h + 1) * NOPE] for h in heads]
            + [wv[:, h * VD : (h + 1) * VD] for h in heads],
            axis=1,
        )  # [KVR, 512]
        wproj_core = np.concatenate(
            [w_proj[h * VD : (h + 1) * VD, :] for h in heads], axis=0
        )  # [256, D]
        in_maps.append(
            {
                "xTw": np.ascontiguousarray(x[b].T[:, W * g : W * (g + 1)]).astype(NBF),
                "cropew": np.ascontiguousarray(crope[0:ROPE, W * g : W * (g + 1)]),
                "wcq": w_cq.astype(NBF),
                "wckvkr": wckvkr.astype(NBF),
                "wq": wq_core.astype(NBF),
                "wkv": wkv_core.astype(NBF),
                "wproj": wproj_core.astype(NBF),
                "crope": crope,
                "masks": masks,
            }
        )

    res = run_bass_kernel_spmd(nc, in_maps, list(range(NCORES)))
    LAST_RESULT = res
    outs = [np.asarray(r["out"], np.float32) for r in res.results]
    gpb = NCORES // B
    out = np.stack(
        [sum(outs[b * gpb + g] for g in range(gpb)) for b in range(B)], axis=0
    )
    return out



# revision 15
# speedup vs baseline: 1.0224x; 1.0224x over previous
"""MLA (multi-head latent attention) forward on 8 TRN2 NeuronCores.

Sharding: core = 4*b + g  (b = batch 0..1, g = head-group 0..3, 4 heads each).
Each core compresses its own 512-token window, RMS-normalizes the latents
in place (rsqrt folded into the latents before the AllGather), gathers the
latents within its batch group, decompresses its 4 heads, runs causal
attention over the full 2048 tokens with 128-granular triangular slicing,
and projects to a bf16 partial output.  Host sums the 4 partials per batch.

Key layout choices:
 - latents ship normalized: no per-window rsqrt rows / broadcasts downstream
 - qT/kT are [96, 2048] per head; attention loops key-block (kc) outer with
   query start at 128*kc, chunked at the 1024-column half boundary
 - softmax denominator comes from a ones-column in V; its reciprocal is
   exp(-ln(d)) on the scalar engine (Ln+Exp share an ACT table)
 - score->exp->PV is software-pipelined (PV emitted one chunk behind)
 - collectives: tiny warmup rendezvous first, then ckv+kr gather, then the
   cq gather in two halves so q-decompress can start on the first half
"""

import sys

sys.path.insert(0, "/opt/trn_rl_repo")

import numpy as np
import ml_dtypes

from concourse import bacc, bass, bass_isa, mybir, tile
from concourse.bass_utils import run_bass_kernel_spmd

# problem dims (hardcoded per contract)
B, S, D = 2, 2048, 2048
H = 16
NOPE, ROPE, VD = 64, 32, 64
QR, KVR = 768, 256
EPS = 1e-6
THETA = 10000.0

HG = 4  # heads per core
NCORES = 8
P = 128
W = 512  # own-token window
NW = S // W  # 4
QKD = NOPE + ROPE  # 96
HALF = 1024  # query half (psum-bank limited chunk)
CKR = KVR + ROPE  # 288 rows in gather A
NKC = S // P  # 16 key blocks

BF = mybir.dt.bfloat16
F32 = mybir.dt.float32
NBF = ml_dtypes.bfloat16
MULT = mybir.AluOpType.mult
AFT = mybir.ActivationFunctionType

LAST_RESULT = None
_CACHE = {}


def _build_nc():
    nc = bacc.Bacc("TRN2", debug=False)
    with tile.TileContext(nc) as tc:
        with (
            tc.tile_pool(name="dram", bufs=1, space="DRAM") as dram,
            tc.tile_pool(name="wres", bufs=1) as wres,
            tc.tile_pool(name="lat", bufs=1) as lat,
            tc.tile_pool(name="xin", bufs=1) as xin,
            tc.tile_pool(name="stg", bufs=2) as stg,
            tc.tile_pool(name="sqa", bufs=2) as sqa,
            tc.tile_pool(name="row", bufs=2) as rowp,
            tc.tile_pool(name="pt", bufs=3) as ptp,
            tc.tile_pool(name="rbc", bufs=2) as rbcp,
            tc.tile_pool(name="ost", bufs=3) as ostp,
            tc.tile_pool(name="psA", bufs=2, space="PSUM") as psA,
            tc.tile_pool(name="psB", bufs=2, space="PSUM") as psB,
        ):
            # ---------------- DRAM params ----------------
            xTw = dram.tile([D, W], BF, kind="ExternalInput", name="xTw", uniquify=False)
            wcq = dram.tile([D, QR], BF, kind="ExternalInput", name="wcq", uniquify=False)
            wckvkr = dram.tile(
                [D, CKR], BF, kind="ExternalInput", name="wckvkr", uniquify=False
            )
            wq = dram.tile(
                [QR, HG * QKD], BF, kind="ExternalInput", name="wq", uniquify=False
            )
            wkv = dram.tile(
                [KVR, HG * (NOPE + VD)], BF, kind="ExternalInput", name="wkv",
                uniquify=False,
            )
            wproj = dram.tile(
                [HG * VD, D], BF, kind="ExternalInput", name="wproj", uniquify=False
            )
            cropeq_d = dram.tile(
                [QKD, S], BF, kind="ExternalInput", name="cropeq", uniquify=False
            )
            cropew_d = dram.tile(
                [ROPE, W], BF, kind="ExternalInput", name="cropew", uniquify=False
            )
            mask_d = dram.tile(
                [P, P], BF, kind="ExternalInput", name="mask", uniquify=False
            )
            out_d = dram.tile([S, D], BF, kind="ExternalOutput", name="out", uniquify=False)

            # collective buffers
            warm_i = dram.tile([1, 16], BF, kind="Internal", name="warm_i", uniquify=False)
            warm_o = dram.tile([4, 16], BF, kind="Internal", name="warm_o", uniquify=False)
            cc_in = dram.tile([CKR + QR, W], BF, kind="Internal", name="cc_in", uniquify=False)
            cc_oa = dram.tile([NW, CKR, W], BF, kind="Internal", name="cc_oa", uniquify=False)
            cc_ob1 = dram.tile([NW, 3 * P, W], BF, kind="Internal", name="cc_ob1", uniquify=False)
            cc_ob2 = dram.tile([NW, 3 * P, W], BF, kind="Internal", name="cc_ob2", uniquify=False)

            GROUPS = [[0, 1, 2, 3], [4, 5, 6, 7]]

            def gather(in_ap, out_ap):
                nc.gpsimd.collective_compute(
                    "AllGather",
                    mybir.AluOpType.bypass,
                    replica_groups=GROUPS,
                    ins=[in_ap],
                    outs=[out_ap],
                )

            # ---------------- warmup rendezvous ----------------
            wt = rowp.tile([1, 16], BF, tag="warm")
            nc.vector.memset(wt[:], 0.0)
            nc.sync.dma_start(out=warm_i[:], in_=wt[:])
            gather(warm_i[:], warm_o[:])

            # ---------------- resident SBUF ----------------
            # x staging shares its slot with cqT (x dies before cqT fills)
            NC_ = D // P  # 16 contraction chunks
            x_sb = xin.tile([P, NC_, W], BF, tag="big")
            wckvkr_sb = wres.tile([P, NC_, CKR], BF, tag="wckvkr")
            wcq_sb = wres.tile([P, NC_, QR], BF, tag="wcq")
            wq_sb = wres.tile([P, QR // P, HG * QKD], BF, tag="wq")
            wkv_sb = wres.tile([P, KVR // P, HG * (NOPE + VD)], BF, tag="wkv")
            wproj_sb = wres.tile([P, (HG * VD) // P, D], BF, tag="wproj")
            cropeq_sb = wres.tile([QKD, S], BF, tag="cropeq")
            cropew_sb = wres.tile([ROPE, W], BF, tag="cropew")
            mask_sb = wres.tile([P, P], BF, tag="mask")

            # x + compression weights, interleaved 2-chunk DMAs (pipelined MMs)
            for c2 in range(NC_ // 2):
                sl = slice(2 * c2 * P, (2 * c2 + 2) * P)
                nc.sync.dma_start(
                    out=x_sb[:, 2 * c2 : 2 * c2 + 2, :],
                    in_=xTw[sl, :].rearrange("(c p) w -> p c w", p=P),
                )
                nc.sync.dma_start(
                    out=wckvkr_sb[:, 2 * c2 : 2 * c2 + 2, :],
                    in_=wckvkr[sl, :].rearrange("(c p) k -> p c k", p=P),
                )
            for c2 in range(NC_ // 2):
                sl = slice(2 * c2 * P, (2 * c2 + 2) * P)
                nc.sync.dma_start(
                    out=wcq_sb[:, 2 * c2 : 2 * c2 + 2, :],
                    in_=wcq[sl, :].rearrange("(c p) q -> p c q", p=P),
                )
            # decompress/attention weights (scalar queue; needed later)
            nc.scalar.dma_start(
                out=wq_sb[:],
                in_=wq[:].rearrange("(c p) k -> p c k", p=P),
            )
            nc.scalar.dma_start(
                out=wkv_sb[:],
                in_=wkv[:].rearrange("(c p) k -> p c k", p=P),
            )
            nc.scalar.dma_start(out=cropeq_sb[:], in_=cropeq_d[:])
            nc.scalar.dma_start(out=cropew_sb[:], in_=cropew_d[:])
            nc.scalar.dma_start(out=mask_sb[:], in_=mask_d[:])
            nc.scalar.dma_start(
                out=wproj_sb[:],
                in_=wproj[:].rearrange("(c p) k -> p c k", p=P),
            )

            # eps row for the rsqrt (bias APs must be [P,1] SBUF)
            cb = wres.tile([P, 2], F32, tag="cb")
            nc.vector.memset(cb[:, 0:1], EPS)

            # ---------------- compression (own window) ----------------
            # normalized-latent staging
            ckv_n = stg.tile([P, KVR // P, W], BF, tag="ckvn", bufs=1)
            cq_n = stg.tile([P, QR // P, W], BF, tag="cqn", bufs=1)
            kr_n = stg.tile([ROPE, W], BF, tag="krn", bufs=1)

            def compress_group(n_m, col0, raw, acc, scale):
                """matmul n_m m-tiles starting at weight col0 into raw bf16
                staging + accumulate squares; returns after emitting MMs."""
                for m in range(n_m):
                    ps = psA.tile([P, W], F32, tag="ps")
                    for c in range(NC_):
                        nc.tensor.matmul(
                            ps[:],
                            (wckvkr_sb if scale == "kv" else wcq_sb)[
                                :, c, col0 + m * P : col0 + (m + 1) * P
                            ],
                            x_sb[:, c, :],
                            start=(c == 0),
                            stop=(c == NC_ - 1),
                        )
                    nc.vector.tensor_copy(out=raw[:, m, :], in_=ps[:])
                    sq = sqa.tile([P, W], BF, tag="sq")
                    nc.scalar.square(out=sq[:], in_=ps[:])
                    if m == 0:
                        nc.vector.tensor_copy(out=acc[:], in_=sq[:])
                    else:
                        nc.vector.tensor_add(out=acc[:], in0=acc[:], in1=sq[:])

            def rsqrt_bcast(acc, inv_n):
                red = sqa.tile([P, W], F32, tag="red")
                nc.gpsimd.partition_all_reduce(
                    red[:], acc[:], channels=P, reduce_op=bass_isa.ReduceOp.add
                )
                srow = rowp.tile([1, W], F32, tag="srow")
                nc.scalar.activation(
                    out=srow[:], in_=red[0:1, :], func=AFT.Sqrt,
                    bias=cb[0:1, 0:1], scale=inv_n,
                )
                rrow = rowp.tile([1, W], F32, tag="rrow")
                nc.vector.reciprocal(out=rrow[:], in_=srow[:])
                rbc = sqa.tile([P, W], F32, tag="rbc")
                nc.gpsimd.partition_broadcast(rbc[:], rrow[:])
                return rbc

            # --- ckv (2 m-tiles) + kr ---
            acc_kv = sqa.tile([P, W], F32, tag="acckv", bufs=1)
            compress_group(KVR // P, 0, ckv_n, acc_kv, "kv")
            pkr = psA.tile([ROPE, W], F32, tag="ps")
            for c in range(NC_):
                nc.tensor.matmul(
                    pkr[:],
                    wckvkr_sb[:, c, KVR:CKR],
                    x_sb[:, c, :],
                    start=(c == 0),
                    stop=(c == NC_ - 1),
                )
            nc.vector.tensor_tensor(out=kr_n[:], in0=pkr[:], in1=cropew_sb[:], op=MULT)
            nc.sync.dma_start(out=cc_in[KVR:CKR, :], in_=kr_n[:])
            rbc_kv = rsqrt_bcast(acc_kv, 1.0 / KVR)
            for m in range(KVR // P):
                nc.vector.tensor_tensor(
                    out=ckv_n[:, m, :], in0=ckv_n[:, m, :], in1=rbc_kv[:], op=MULT
                )
            nc.sync.dma_start(
                out=cc_in[0:KVR, :].rearrange("(m p) w -> p m w", p=P), in_=ckv_n[:]
            )
            gather(cc_in[0:CKR, :], cc_oa[:])

            # --- cq (6 m-tiles, gathered in two halves) ---
            acc_q = sqa.tile([P, W], F32, tag="accq", bufs=1)
            compress_group(QR // P, 0, cq_n, acc_q, "q")
            rbc_q = rsqrt_bcast(acc_q, 1.0 / QR)
            for m in range(3):
                nc.vector.tensor_tensor(
                    out=cq_n[:, m, :], in0=cq_n[:, m, :], in1=rbc_q[:], op=MULT
                )
            nc.sync.dma_start(
                out=cc_in[CKR : CKR + 3 * P, :].rearrange("(m p) w -> p m w", p=P),
                in_=cq_n[:, 0:3, :],
            )
            gather(cc_in[CKR : CKR + 3 * P, :], cc_ob1[:])
            for m in range(3, 6):
                nc.vector.tensor_tensor(
                    out=cq_n[:, m, :], in0=cq_n[:, m, :], in1=rbc_q[:], op=MULT
                )
            nc.sync.dma_start(
                out=cc_in[CKR + 3 * P : CKR + QR, :].rearrange("(m p) w -> p m w", p=P),
                in_=cq_n[:, 3:6, :],
            )
            gather(cc_in[CKR + 3 * P : CKR + QR, :], cc_ob2[:])

            # ---------------- latent tiles (full sequence) ----------------
            ckvT = lat.tile([P, KVR // P, S], BF, tag="ckvT")
            krT = lat.tile([ROPE, S], BF, tag="krT")
            cqT = xin.tile([P, QR // P, S], BF, tag="big")  # reuses x slot

            # fill every window from the gathered output (the program is
            # SPMD-identical across cores, so there is no per-core "own
            # window" specialization; the gather returns our shard too)
            for w in range(NW):
                ws = slice(w * W, (w + 1) * W)
                nc.sync.dma_start(
                    out=ckvT[:, :, ws],
                    in_=cc_oa[w, 0:KVR, :].rearrange("(m p) w -> p m w", p=P),
                )
                nc.sync.dma_start(out=krT[:, ws], in_=cc_oa[w, KVR:CKR, :])
                nc.sync.dma_start(
                    out=cqT[:, 0:3, ws],
                    in_=cc_ob1[w, :, :].rearrange("(m p) w -> p m w", p=P),
                )
                nc.sync.dma_start(
                    out=cqT[:, 3:6, ws],
                    in_=cc_ob2[w, :, :].rearrange("(m p) w -> p m w", p=P),
                )

            # ---------------- decompress ----------------
            kT = [lat.tile([QKD, S], BF, tag=f"kT{h}", name=f"kT{h}") for h in range(HG)]
            qT = [lat.tile([QKD, S], BF, tag=f"qT{h}", name=f"qT{h}") for h in range(HG)]
            vaug = lat.tile([P, NKC, HG, VD + 1], BF, tag="vaug")
            oT = lat.tile([P, 2, S], BF, tag="oT")
            nc.vector.memset(vaug[:, :, :, VD : VD + 1], 1.0)

            # k_nope per (head, half): psum [64, HALF]
            for h in range(HG):
                for q2 in range(S // HALF):
                    hs = slice(q2 * HALF, (q2 + 1) * HALF)
                    ps = psA.tile([NOPE, HALF], F32, tag="ps")
                    for r in range(KVR // P):
                        for s2 in range(2):
                            nc.tensor.matmul(
                                ps[:, s2 * W : (s2 + 1) * W],
                                wkv_sb[:, r, h * NOPE : (h + 1) * NOPE],
                                ckvT[:, r, q2 * HALF + s2 * W : q2 * HALF + (s2 + 1) * W],
                                start=(r == 0),
                                stop=(r == KVR // P - 1),
                            )
                    if q2 == 0:
                        nc.vector.tensor_copy(out=kT[h][0:NOPE, hs], in_=ps[:])
                    else:
                        nc.scalar.copy(out=kT[h][0:NOPE, hs], in_=ps[:])
                nc.vector.tensor_copy(out=kT[h][NOPE:QKD, :], in_=krT[:])

            # v token-major: psum [128 tokens, HG*VD]
            for ck in range(NKC):
                ps = psA.tile([P, HG * VD], F32, tag="ps")
                for r in range(KVR // P):
                    nc.tensor.matmul(
                        ps[:],
                        ckvT[:, r, ck * P : (ck + 1) * P],
                        wkv_sb[:, r, HG * NOPE : HG * (NOPE + VD)],
                        start=(r == 0),
                        stop=(r == KVR // P - 1),
                    )
                nc.scalar.activation(
                    out=vaug[:, ck, :, 0:VD],
                    in_=ps[:].rearrange("p (h d) -> p h d", h=HG),
                    func=AFT.Copy,
                )

            # q per (head, half): psum [96, HALF]; rope rows get cos at evict
            for h in range(HG):
                for q2 in range(S // HALF):
                    hs = slice(q2 * HALF, (q2 + 1) * HALF)
                    ps = psA.tile([QKD, HALF], F32, tag="ps")
                    for r in range(QR // P):
                        for s2 in range(2):
                            nc.tensor.matmul(
                                ps[:, s2 * W : (s2 + 1) * W],
                                wq_sb[:, r, h * QKD : (h + 1) * QKD],
                                cqT[:, r, q2 * HALF + s2 * W : q2 * HALF + (s2 + 1) * W],
                                start=(r == 0),
                                stop=(r == QR // P - 1),
                            )
                    nc.vector.tensor_tensor(
                        out=qT[h][:, hs], in0=ps[:], in1=cropeq_sb[:, hs], op=MULT
                    )

            # ---------------- attention ----------------
            # per head: kc-outer, query start at 128*kc, chunks split at the
            # HALF boundary; PV emitted one chunk behind (sw pipeline).
            for h in range(HG):
                opv = [
                    psB.tile([VD + 1, HALF], F32, tag="opv", name=f"opv{h}_{i}")
                    for i in range(2)
                ]
                chunks = []  # (kc, qh, q0, q1)
                for kc in range(NKC):
                    for qh in range(2):
                        q0 = max(P * kc, qh * HALF)
                        q1 = (qh + 1) * HALF
                        if q0 >= q1:
                            continue
                        chunks.append((kc, qh, q0, q1))
                pending = []

                def pieces(q0, q1):
                    # split [q0,q1) at absolute 512-column boundaries
                    out = []
                    a = q0
                    while a < q1:
                        b = min((a // W + 1) * W, q1)
                        out.append((a, b))
                        a = b
                    return out

                def flush_pv(n):
                    while len(pending) > n:
                        kc_, qh_, q0_, q1_, pt_ = pending.pop(0)
                        for a, b in pieces(q0_, q1_):
                            nc.tensor.matmul(
                                opv[qh_][:, a - qh_ * HALF : b - qh_ * HALF],
                                vaug[:, kc_, h, :],
                                pt_[:, a - q0_ : b - q0_],
                                start=(kc_ == 0),
                                stop=(kc_ == (qh_ + 1) * (NKC // 2) - 1),
                            )

                for (kc, qh, q0, q1) in chunks:
                    # psum offsets are half-absolute so pieces stay inside banks
                    sp = psA.tile([P, HALF], F32, tag="ps")
                    o0 = q0 - qh * HALF
                    for a, b in pieces(q0, q1):
                        nc.tensor.matmul(
                            sp[:, a - qh * HALF : b - qh * HALF],
                            kT[h][:, kc * P : (kc + 1) * P],
                            qT[h][:, a:b],
                            start=True,
                            stop=True,
                        )
                    pt = ptp.tile([P, HALF], BF, tag="pt")
                    nc.scalar.activation(
                        out=pt[:, 0 : q1 - q0],
                        in_=sp[:, o0 : q1 - qh * HALF],
                        func=AFT.Exp,
                    )
                    if q0 == P * kc:  # diagonal chunk: mask first 128 cols
                        nc.vector.tensor_tensor(
                            out=pt[:, 0:P], in0=pt[:, 0:P], in1=mask_sb[:], op=MULT
                        )
                    pending.append((kc, qh, q0, q1, pt))
                    flush_pv(1)
                flush_pv(0)

                # normalize: rec = exp(-ln(denom)), broadcast, multiply
                for qh in range(2):
                    lnr = rowp.tile([1, HALF], F32, tag="lnr")
                    nc.scalar.activation(
                        out=lnr[:], in_=opv[qh][VD : VD + 1, :], func=AFT.Ln
                    )
                    rec = rowp.tile([1, HALF], F32, tag="lnr")
                    nc.scalar.activation(out=rec[:], in_=lnr[:], func=AFT.Exp, scale=-1.0)
                    rbc = rbcp.tile([VD, HALF], F32, tag="rbc")
                    nc.gpsimd.partition_broadcast(rbc[:], rec[:])
                    oTh = oT[VD * (h % 2) : VD * (h % 2) + VD, h // 2, :]
                    nc.vector.tensor_tensor(
                        out=oTh[:, qh * HALF : (qh + 1) * HALF],
                        in0=opv[qh][0:VD, :],
                        in1=rbc[:],
                        op=MULT,
                    )

            # ---------------- projection ----------------
            for t in range(S // P):
                pps = []
                for wc in range(2):
                    pp = psB.tile([P, HALF], F32, tag="opv")
                    for i in range(2):
                        for s2 in range(2):
                            nc.tensor.matmul(
                                pp[:, s2 * W : (s2 + 1) * W],
                                oT[:, i, t * P : (t + 1) * P],
                                wproj_sb[:, i, wc * HALF + s2 * W : wc * HALF + (s2 + 1) * W],
                                start=(i == 0),
                                stop=(i == 1),
                            )
                    pps.append(pp)
                for wc, pp in enumerate(pps):
                    o = ostp.tile([P, HALF], BF, tag="ost")
                    if wc == 0:
                        nc.vector.tensor_copy(out=o[:], in_=pp[:])
                    else:
                        nc.scalar.copy(out=o[:], in_=pp[:])
                    nc.sync.dma_start(
                        out=out_d[t * P : (t + 1) * P, wc * HALF : (wc + 1) * HALF],
                        in_=o[:],
                    )

    nc.compile()
    return nc


def _rope_fold():
    """32x32 butterfly for RoPE with the reference's sin==cos bug."""
    Bm = np.zeros((ROPE, ROPE), np.float32)
    for j in range(ROPE // 2):
        Bm[2 * j, 2 * j] = 1.0
        Bm[2 * j, 2 * j + 1] = -1.0
        Bm[2 * j + 1, 2 * j] = 1.0
        Bm[2 * j + 1, 2 * j + 1] = 1.0
    return Bm


def _host_tables():
    freqs = 1.0 / (THETA ** (np.arange(0, ROPE, 2, dtype=np.float32) / ROPE))
    ang = np.outer(np.arange(S, dtype=np.float32), freqs)  # [S, 16]
    cos = np.cos(ang)
    crope32 = np.repeat(cos, 2, axis=1).T.copy()  # [32, S]
    cropeq = np.concatenate(
        [np.ones((NOPE, S), np.float32), crope32], axis=0
    )  # [96, S]
    mask = np.zeros((P, P), np.float32)
    for k in range(P):
        mask[k, k:] = 1.0
    return cropeq.astype(NBF), crope32.astype(NBF), mask.astype(NBF)


def kernel(**inputs):
    global LAST_RESULT
    x = np.asarray(inputs["x"], np.float32)
    w_cq = np.asarray(inputs["w_cq"], np.float32)
    w_q_nope = np.asarray(inputs["w_q_nope"], np.float32)
    w_q_rope = np.asarray(inputs["w_q_rope"], np.float32)
    q_g = np.asarray(inputs["q_g"], np.float32)
    w_ckv = np.asarray(inputs["w_ckv"], np.float32)
    w_k_nope = np.asarray(inputs["w_k_nope"], np.float32)
    w_v = np.asarray(inputs["w_v"], np.float32)
    kv_g = np.asarray(inputs["kv_g"], np.float32)
    w_k_rope = np.asarray(inputs["w_k_rope"], np.float32)
    w_proj = np.asarray(inputs["w_proj"], np.float32)

    Bm = _rope_fold()
    cropeq, crope32, mask = _host_tables()
    scale = 1.0 / np.sqrt(QKD)

    wqn = w_q_nope * q_g[:, None] * scale  # [QR, H*64]
    wqr = w_q_rope * q_g[:, None] * scale  # [QR, H*32]
    wkn = w_k_nope * kv_g[:, None]  # [KVR, H*64]
    wv = w_v * kv_g[:, None]  # [KVR, H*64]
    wkr = (w_k_rope @ Bm.T) / H  # [D, 32]
    wckvkr = np.concatenate([w_ckv, wkr], axis=1)  # [D, 288]

    if "nc" not in _CACHE:
        _CACHE["nc"] = _build_nc()
    nc = _CACHE["nc"]

    in_maps = []
    for core in range(NCORES):
        b, g = divmod(core, NCORES // B)
        heads = range(HG * g, HG * (g + 1))
        wq_cols = []
        for h in heads:
            wq_cols.append(wqn[:, h * NOPE : (h + 1) * NOPE])
            wq_cols.append(wqr[:, h * ROPE : (h + 1) * ROPE] @ Bm.T)
        wq_core = np.concatenate(wq_cols, axis=1)  # [QR, 384]
        wkv_core = np.concatenate(
            [wkn[:, h * NOPE : (h + 1) * NOPE] for h in heads]
            + [wv[:, h * VD : (h + 1) * VD] for h in heads],
            axis=1,
        )  # [KVR, 512]
        wproj_core = np.concatenate(
            [w_proj[h * VD : (h + 1) * VD, :] for h in heads], axis=0
        )  # [256, D]
        in_maps.append(
            {
                "xTw": np.ascontiguousarray(x[b].T[:, W * g : W * (g + 1)]).astype(NBF),
                "cropew": np.ascontiguousarray(crope32[:, W * g : W * (g + 1)]),
                "wcq": w_cq.astype(NBF),
                "wckvkr": wckvkr.astype(NBF),
                "wq": wq_core.astype(NBF),
                "wkv": wkv_core.astype(NBF),
                "wproj": wproj_core.astype(NBF),
                "cropeq": cropeq,
                "mask": mask,
            }
        )

    res = run_bass_kernel_spmd(nc, in_maps, list(range(NCORES)))
    LAST_RESULT = res
    outs = [np.asarray(r["out"], np.float32) for r in res.results]
    gpb = NCORES // B
    out = np.stack(
        [sum(outs[b * gpb + g] for g in range(gpb)) for b in range(B)], axis=0
    )
    return out


# revision 22
# speedup vs baseline: 1.0652x; 1.0419x over previous
"""MLA (multi-head latent attention) forward on 8 TRN2 NeuronCores.

Sharding: core = 4*b + g  (b = batch 0..1, g = head-group 0..3, 4 heads each).
Each core compresses its own 512-token window, RMS-normalizes the latents
in place (rsqrt folded into the latents before the AllGather), gathers the
latents within its batch group, decompresses its 4 heads, runs causal
attention over the full 2048 tokens with 128-granular triangular slicing,
and projects to a bf16 partial output.  Host sums the 4 partials per batch.

Key layout choices:
 - latents ship normalized: no per-window rsqrt rows / broadcasts downstream
 - qT/kT are [96, 2048] per head; attention loops key-block (kc) outer with
   query start at 128*kc, chunked at the 1024-column half boundary
 - softmax denominator comes from a ones-column in V; its reciprocal is
   exp(-ln(d)) on the scalar engine (Ln+Exp share an ACT table)
 - score->exp->PV is software-pipelined (PV emitted one chunk behind)
 - collectives: tiny warmup rendezvous first, then ckv+kr gather, then the
   cq gather in two halves so q-decompress can start on the first half
"""

import sys

sys.path.insert(0, "/opt/trn_rl_repo")

import numpy as np
import ml_dtypes

from concourse import bacc, bass, bass_isa, mybir, tile
from concourse.bass_utils import run_bass_kernel_spmd

# problem dims (hardcoded per contract)
B, S, D = 2, 2048, 2048
H = 16
NOPE, ROPE, VD = 64, 32, 64
QR, KVR = 768, 256
EPS = 1e-6
THETA = 10000.0

HG = 4  # heads per core
NCORES = 8
P = 128
W = 512  # own-token window
NW = S // W  # 4
QKD = NOPE + ROPE  # 96
HALF = 1024  # query half (psum-bank limited chunk)
CKR = KVR + ROPE  # 288 rows in gather A
NKC = S // P  # 16 key blocks

BF = mybir.dt.bfloat16
F32 = mybir.dt.float32
NBF = ml_dtypes.bfloat16
MULT = mybir.AluOpType.mult
AFT = mybir.ActivationFunctionType

LAST_RESULT = None
_CACHE = {}


def _build_nc():
    nc = bacc.Bacc("TRN2", debug=False)
    with tile.TileContext(nc) as tc:
        with (
            tc.tile_pool(name="dram", bufs=1, space="DRAM") as dram,
            tc.tile_pool(name="wres", bufs=1) as wres,
            tc.tile_pool(name="lat", bufs=1) as lat,
            tc.tile_pool(name="xin", bufs=1) as xin,
            tc.tile_pool(name="stg", bufs=2) as stg,
            tc.tile_pool(name="sqa", bufs=2) as sqa,
            tc.tile_pool(name="row", bufs=2) as rowp,
            tc.tile_pool(name="pt", bufs=3) as ptp,
            tc.tile_pool(name="rbc", bufs=2) as rbcp,
            tc.tile_pool(name="ost", bufs=3) as ostp,
            tc.tile_pool(name="psA", bufs=2, space="PSUM") as psA,
            tc.tile_pool(name="psB", bufs=2, space="PSUM") as psB,
        ):
            # ---------------- DRAM params ----------------
            xTw = dram.tile([D, W], BF, kind="ExternalInput", name="xTw", uniquify=False)
            wcq = dram.tile([D, QR], BF, kind="ExternalInput", name="wcq", uniquify=False)
            wckvkr = dram.tile(
                [D, CKR], BF, kind="ExternalInput", name="wckvkr", uniquify=False
            )
            wq = dram.tile(
                [QR, HG * QKD], BF, kind="ExternalInput", name="wq", uniquify=False
            )
            wkv = dram.tile(
                [KVR, HG * (NOPE + VD)], BF, kind="ExternalInput", name="wkv",
                uniquify=False,
            )
            wproj = dram.tile(
                [HG * VD, D], BF, kind="ExternalInput", name="wproj", uniquify=False
            )
            cropeq_d = dram.tile(
                [QKD, S], BF, kind="ExternalInput", name="cropeq", uniquify=False
            )
            cropew_d = dram.tile(
                [ROPE, W], BF, kind="ExternalInput", name="cropew", uniquify=False
            )
            mask_d = dram.tile(
                [P, P], BF, kind="ExternalInput", name="mask", uniquify=False
            )
            out_d = dram.tile([S, D], BF, kind="ExternalOutput", name="out", uniquify=False)

            # collective buffers
            warm_i = dram.tile([1, 16], BF, kind="Internal", name="warm_i", uniquify=False)
            warm_o = dram.tile([4, 16], BF, kind="Internal", name="warm_o", uniquify=False)
            cc_in = dram.tile([CKR + QR, W], BF, kind="Internal", name="cc_in", uniquify=False)
            cc_oa = dram.tile([NW, CKR, W], BF, kind="Internal", name="cc_oa", uniquify=False)
            cc_ob = dram.tile([NW, QR, W], BF, kind="Internal", name="cc_ob", uniquify=False)

            GROUPS = [[0, 1, 2, 3], [4, 5, 6, 7]]

            def gather(in_ap, out_ap):
                nc.gpsimd.collective_compute(
                    "AllGather",
                    mybir.AluOpType.bypass,
                    replica_groups=GROUPS,
                    ins=[in_ap],
                    outs=[out_ap],
                )

            # ---------------- warmup rendezvous ----------------
            # gather garbage with zero dependencies: the first collective
            # carries the cross-core rendezvous barrier, so fire it at t=0
            # and let it absorb kickoff skew under the compression phase
            gather(warm_i[:], warm_o[:])

            # ---------------- resident SBUF ----------------
            # x staging shares its slot with cqT (x dies before cqT fills)
            NC_ = D // P  # 16 contraction chunks
            x_sb = xin.tile([P, NC_, W], BF, tag="big")
            wckvkr_sb = wres.tile([P, NC_, CKR], BF, tag="wckvkr")
            wcq_sb = wres.tile([P, NC_, QR], BF, tag="wcq")
            wq_sb = wres.tile([P, QR // P, HG * QKD], BF, tag="wq")
            wkv_sb = wres.tile([P, KVR // P, HG * (NOPE + VD)], BF, tag="wkv")
            wproj_sb = wres.tile([P, (HG * VD) // P, D], BF, tag="wproj")
            cropeq_sb = wres.tile([QKD, S], BF, tag="cropeq")
            cropew_sb = wres.tile([ROPE, W], BF, tag="cropew")
            mask_sb = wres.tile([P, P], BF, tag="mask")

            # x + compression weights, interleaved 2-chunk DMAs (pipelined MMs)
            # cq is compressed first, so wcq rides with x up front
            for c2 in range(NC_ // 2):
                sl = slice(2 * c2 * P, (2 * c2 + 2) * P)
                nc.sync.dma_start(
                    out=x_sb[:, 2 * c2 : 2 * c2 + 2, :],
                    in_=xTw[sl, :].rearrange("(c p) w -> p c w", p=P),
                )
                nc.sync.dma_start(
                    out=wcq_sb[:, 2 * c2 : 2 * c2 + 2, :],
                    in_=wcq[sl, :].rearrange("(c p) q -> p c q", p=P),
                )
            for c2 in range(NC_ // 2):
                sl = slice(2 * c2 * P, (2 * c2 + 2) * P)
                nc.sync.dma_start(
                    out=wckvkr_sb[:, 2 * c2 : 2 * c2 + 2, :],
                    in_=wckvkr[sl, :].rearrange("(c p) k -> p c k", p=P),
                )
            # decompress/attention weights (scalar queue; needed later)
            nc.scalar.dma_start(
                out=wq_sb[:],
                in_=wq[:].rearrange("(c p) k -> p c k", p=P),
            )
            nc.scalar.dma_start(
                out=wkv_sb[:],
                in_=wkv[:].rearrange("(c p) k -> p c k", p=P),
            )
            nc.scalar.dma_start(out=cropeq_sb[:], in_=cropeq_d[:])
            nc.scalar.dma_start(out=cropew_sb[:], in_=cropew_d[:])
            nc.scalar.dma_start(out=mask_sb[:], in_=mask_d[:])
            nc.scalar.dma_start(
                out=wproj_sb[:],
                in_=wproj[:].rearrange("(c p) k -> p c k", p=P),
            )

            # eps row for the rsqrt (bias APs must be [P,1] SBUF)
            cb = wres.tile([P, 2], F32, tag="cb")
            nc.vector.memset(cb[:, 0:1], EPS)

            # ---------------- compression (own window) ----------------
            # normalized-latent staging
            ckv_n = stg.tile([P, KVR // P, W], BF, tag="ckvn", bufs=1)
            cq_n = stg.tile([P, QR // P, W], BF, tag="cqn", bufs=1)
            kr_n = stg.tile([ROPE, W], BF, tag="krn", bufs=1)

            def compress_group(n_m, col0, raw, acc, scale):
                """matmul n_m m-tiles starting at weight col0 into raw bf16
                staging + accumulate squares; returns after emitting MMs."""
                for m in range(n_m):
                    ps = psA.tile([P, W], F32, tag="ps")
                    for c in range(NC_):
                        nc.tensor.matmul(
                            ps[:],
                            (wckvkr_sb if scale == "kv" else wcq_sb)[
                                :, c, col0 + m * P : col0 + (m + 1) * P
                            ],
                            x_sb[:, c, :],
                            start=(c == 0),
                            stop=(c == NC_ - 1),
                        )
                    nc.vector.tensor_copy(out=raw[:, m, :], in_=ps[:])
                    sq = sqa.tile([P, W], BF, tag="sq")
                    nc.scalar.square(out=sq[:], in_=ps[:])
                    if m == 0:
                        nc.vector.tensor_copy(out=acc[:], in_=sq[:])
                    else:
                        nc.vector.tensor_add(out=acc[:], in0=acc[:], in1=sq[:])

            def rsqrt_bcast(acc, inv_n):
                red = sqa.tile([P, W], F32, tag="red")
                nc.gpsimd.partition_all_reduce(
                    red[:], acc[:], channels=P, reduce_op=bass_isa.ReduceOp.add
                )
                srow = rowp.tile([1, W], F32, tag="srow")
                nc.scalar.activation(
                    out=srow[:], in_=red[0:1, :], func=AFT.Sqrt,
                    bias=cb[0:1, 0:1], scale=inv_n,
                )
                rrow = rowp.tile([1, W], F32, tag="rrow")
                nc.vector.reciprocal(out=rrow[:], in_=srow[:])
                rbc = sqa.tile([P, W], F32, tag="rbc")
                nc.gpsimd.partition_broadcast(rbc[:], rrow[:])
                return rbc

            # --- cq first (6 m-tiles): its gather gates the long q-decompress,
            # so it goes on the collective stream ahead of ckv ---
            acc_q = sqa.tile([P, W], F32, tag="accq", bufs=1)
            compress_group(QR // P, 0, cq_n, acc_q, "q")
            rbc_q = rsqrt_bcast(acc_q, 1.0 / QR)
            for m in range(QR // P):
                nc.vector.tensor_tensor(
                    out=cq_n[:, m, :], in0=cq_n[:, m, :], in1=rbc_q[:], op=MULT
                )
            nc.sync.dma_start(
                out=cc_in[CKR : CKR + QR, :].rearrange("(m p) w -> p m w", p=P),
                in_=cq_n[:],
            )
            gather(cc_in[CKR : CKR + QR, :], cc_ob[:])

            # --- ckv (2 m-tiles) + kr ---
            acc_kv = sqa.tile([P, W], F32, tag="acckv", bufs=1)
            compress_group(KVR // P, 0, ckv_n, acc_kv, "kv")
            pkr = psA.tile([ROPE, W], F32, tag="ps")
            for c in range(NC_):
                nc.tensor.matmul(
                    pkr[:],
                    wckvkr_sb[:, c, KVR:CKR],
                    x_sb[:, c, :],
                    start=(c == 0),
                    stop=(c == NC_ - 1),
                )
            nc.vector.tensor_tensor(out=kr_n[:], in0=pkr[:], in1=cropew_sb[:], op=MULT)
            nc.sync.dma_start(out=cc_in[KVR:CKR, :], in_=kr_n[:])
            rbc_kv = rsqrt_bcast(acc_kv, 1.0 / KVR)
            for m in range(KVR // P):
                nc.vector.tensor_tensor(
                    out=ckv_n[:, m, :], in0=ckv_n[:, m, :], in1=rbc_kv[:], op=MULT
                )
            nc.sync.dma_start(
                out=cc_in[0:KVR, :].rearrange("(m p) w -> p m w", p=P), in_=ckv_n[:]
            )
            gather(cc_in[0:CKR, :], cc_oa[:])

            # ---------------- latent tiles (full sequence) ----------------
            ckvT = lat.tile([P, KVR // P, S], BF, tag="ckvT")
            krT = lat.tile([ROPE, S], BF, tag="krT")
            cqT = xin.tile([P, QR // P, S], BF, tag="big")  # reuses x slot

            # fill every window from the gathered output (the program is
            # SPMD-identical across cores, so there is no per-core "own
            # window" specialization; the gather returns our shard too)
            for w in range(NW):
                ws = slice(w * W, (w + 1) * W)
                nc.sync.dma_start(
                    out=cqT[:, :, ws],
                    in_=cc_ob[w, :, :].rearrange("(m p) w -> p m w", p=P),
                )
            for w in range(NW):
                ws = slice(w * W, (w + 1) * W)
                nc.sync.dma_start(
                    out=ckvT[:, :, ws],
                    in_=cc_oa[w, 0:KVR, :].rearrange("(m p) w -> p m w", p=P),
                )
                nc.sync.dma_start(out=krT[:, ws], in_=cc_oa[w, KVR:CKR, :])

            # ---------------- decompress ----------------
            kT = [lat.tile([QKD, S], BF, tag=f"kT{h}", name=f"kT{h}") for h in range(HG)]
            qT = [lat.tile([QKD, S], BF, tag=f"qT{h}", name=f"qT{h}") for h in range(HG)]
            vaug = lat.tile([P, NKC, HG, VD + 1], BF, tag="vaug")
            oT = lat.tile([P, 2, S], BF, tag="oT")
            nc.vector.memset(vaug[:, :, :, VD : VD + 1], 1.0)

            # q per (head, half): psum [96, HALF]; rope rows get cos at evict.
            # emitted first: gather B (cq) lands before gather A (ckv)
            for h in range(HG):
                for q2 in range(S // HALF):
                    hs = slice(q2 * HALF, (q2 + 1) * HALF)
                    ps = psA.tile([QKD, HALF], F32, tag="ps")
                    for r in range(QR // P):
                        for s2 in range(2):
                            nc.tensor.matmul(
                                ps[:, s2 * W : (s2 + 1) * W],
                                wq_sb[:, r, h * QKD : (h + 1) * QKD],
                                cqT[:, r, q2 * HALF + s2 * W : q2 * HALF + (s2 + 1) * W],
                                start=(r == 0),
                                stop=(r == QR // P - 1),
                            )
                    nc.vector.tensor_tensor(
                        out=qT[h][:, hs], in0=ps[:], in1=cropeq_sb[:, hs], op=MULT
                    )

            # k_nope per (head, half): psum [64, HALF]
            for h in range(HG):
                for q2 in range(S // HALF):
                    hs = slice(q2 * HALF, (q2 + 1) * HALF)
                    ps = psA.tile([NOPE, HALF], F32, tag="ps")
                    for r in range(KVR // P):
                        for s2 in range(2):
                            nc.tensor.matmul(
                                ps[:, s2 * W : (s2 + 1) * W],
                                wkv_sb[:, r, h * NOPE : (h + 1) * NOPE],
                                ckvT[:, r, q2 * HALF + s2 * W : q2 * HALF + (s2 + 1) * W],
                                start=(r == 0),
                                stop=(r == KVR // P - 1),
                            )
                    nc.vector.tensor_copy(out=kT[h][0:NOPE, hs], in_=ps[:])
                nc.vector.tensor_copy(out=kT[h][NOPE:QKD, :], in_=krT[:])

            # v token-major: psum [128 tokens, HG*VD]
            for ck in range(NKC):
                ps = psA.tile([P, HG * VD], F32, tag="ps")
                for r in range(KVR // P):
                    nc.tensor.matmul(
                        ps[:],
                        ckvT[:, r, ck * P : (ck + 1) * P],
                        wkv_sb[:, r, HG * NOPE : HG * (NOPE + VD)],
                        start=(r == 0),
                        stop=(r == KVR // P - 1),
                    )
                nc.vector.tensor_copy(
                    out=vaug[:, ck, :, 0:VD],
                    in_=ps[:].rearrange("p (h d) -> p h d", h=HG),
                )

            # ---------------- attention ----------------
            # per head: kc-outer, query start at 128*kc, chunks split at the
            # HALF boundary; PV emitted one chunk behind (sw pipeline).
            for h in range(HG):
                opv = [
                    psB.tile([VD + 1, HALF], F32, tag="opv", name=f"opv{h}_{i}")
                    for i in range(2)
                ]
                chunks = []  # (kc, qh, q0, q1)
                for kc in range(NKC):
                    for qh in range(2):
                        q0 = max(P * kc, qh * HALF)
                        q1 = (qh + 1) * HALF
                        if q0 >= q1:
                            continue
                        chunks.append((kc, qh, q0, q1))
                pending = []

                def pieces(q0, q1):
                    # split [q0,q1) at absolute 512-column boundaries
                    out = []
                    a = q0
                    while a < q1:
                        b = min((a // W + 1) * W, q1)
                        out.append((a, b))
                        a = b
                    return out

                def flush_pv(n):
                    while len(pending) > n:
                        kc_, qh_, q0_, q1_, pt_ = pending.pop(0)
                        for a, b in pieces(q0_, q1_):
                            nc.tensor.matmul(
                                opv[qh_][:, a - qh_ * HALF : b - qh_ * HALF],
                                vaug[:, kc_, h, :],
                                pt_[:, a - q0_ : b - q0_],
                                start=(kc_ == 0),
                                stop=(kc_ == (qh_ + 1) * (NKC // 2) - 1),
                            )

                for (kc, qh, q0, q1) in chunks:
                    # psum offsets are half-absolute so pieces stay inside banks
                    sp = psA.tile([P, HALF], F32, tag="ps")
                    o0 = q0 - qh * HALF
                    for a, b in pieces(q0, q1):
                        nc.tensor.matmul(
                            sp[:, a - qh * HALF : b - qh * HALF],
                            kT[h][:, kc * P : (kc + 1) * P],
                            qT[h][:, a:b],
                            start=True,
                            stop=True,
                        )
                    pt = ptp.tile([P, HALF], BF, tag="pt")
                    nc.scalar.activation(
                        out=pt[:, 0 : q1 - q0],
                        in_=sp[:, o0 : q1 - qh * HALF],
                        func=AFT.Exp,
                    )
                    if q0 == P * kc:  # diagonal chunk: mask first 128 cols
                        nc.vector.tensor_tensor(
                            out=pt[:, 0:P], in0=pt[:, 0:P], in1=mask_sb[:], op=MULT
                        )
                    pending.append((kc, qh, q0, q1, pt))
                    flush_pv(1)
                flush_pv(0)

                # normalize: rec = exp(-ln(denom)), broadcast, multiply.
                # both Lns then both Exps: two ACT table loads per head, not four
                lnrs = []
                for qh in range(2):
                    lnr = rowp.tile([1, HALF], F32, tag="lnr", name=f"lnr{h}_{qh}")
                    nc.scalar.activation(
                        out=lnr[:], in_=opv[qh][VD : VD + 1, :], func=AFT.Ln
                    )
                    lnrs.append(lnr)
                for qh in range(2):
                    rec = rowp.tile([1, HALF], F32, tag="rec", name=f"rec{h}_{qh}")
                    nc.scalar.activation(
                        out=rec[:], in_=lnrs[qh][:], func=AFT.Exp, scale=-1.0
                    )
                    rbc = rbcp.tile([VD, HALF], F32, tag="rbc")
                    nc.gpsimd.partition_broadcast(rbc[:], rec[:])
                    oTh = oT[VD * (h % 2) : VD * (h % 2) + VD, h // 2, :]
                    nc.vector.tensor_tensor(
                        out=oTh[:, qh * HALF : (qh + 1) * HALF],
                        in0=opv[qh][0:VD, :],
                        in1=rbc[:],
                        op=MULT,
                    )

            # ---------------- projection ----------------
            for t in range(S // P):
                pps = []
                for wc in range(2):
                    pp = psB.tile([P, HALF], F32, tag="opv")
                    for i in range(2):
                        for s2 in range(2):
                            nc.tensor.matmul(
                                pp[:, s2 * W : (s2 + 1) * W],
                                oT[:, i, t * P : (t + 1) * P],
                                wproj_sb[:, i, wc * HALF + s2 * W : wc * HALF + (s2 + 1) * W],
                                start=(i == 0),
                                stop=(i == 1),
                            )
                    pps.append(pp)
                for wc, pp in enumerate(pps):
                    o = ostp.tile([P, HALF], BF, tag="ost")
                    if wc == 0:
                        nc.vector.tensor_copy(out=o[:], in_=pp[:])
                    else:
                        nc.scalar.copy(out=o[:], in_=pp[:])
                    nc.sync.dma_start(
                        out=out_d[t * P : (t + 1) * P, wc * HALF : (wc + 1) * HALF],
                        in_=o[:],
                    )

    nc.compile()
    return nc


def _rope_fold():
    """32x32 butterfly for RoPE with the reference's sin==cos bug."""
    Bm = np.zeros((ROPE, ROPE), np.float32)
    for j in range(ROPE // 2):
        Bm[2 * j, 2 * j] = 1.0
        Bm[2 * j, 2 * j + 1] = -1.0
        Bm[2 * j + 1, 2 * j] = 1.0
        Bm[2 * j + 1, 2 * j + 1] = 1.0
    return Bm


def _host_tables():
    freqs = 1.0 / (THETA ** (np.arange(0, ROPE, 2, dtype=np.float32) / ROPE))
    ang = np.outer(np.arange(S, dtype=np.float32), freqs)  # [S, 16]
    cos = np.cos(ang)
    crope32 = np.repeat(cos, 2, axis=1).T.copy()  # [32, S]
    cropeq = np.concatenate(
        [np.ones((NOPE, S), np.float32), crope32], axis=0
    )  # [96, S]
    mask = np.zeros((P, P), np.float32)
    for k in range(P):
        mask[k, k:] = 1.0
    return cropeq.astype(NBF), crope32.astype(NBF), mask.astype(NBF)


def kernel(**inputs):
    global LAST_RESULT
    x = np.asarray(inputs["x"], np.float32)
    w_cq = np.asarray(inputs["w_cq"], np.float32)
    w_q_nope = np.asarray(inputs["w_q_nope"], np.float32)
    w_q_rope = np.asarray(inputs["w_q_rope"], np.float32)
    q_g = np.asarray(inputs["q_g"], np.float32)
    w_ckv = np.asarray(inputs["w_ckv"], np.float32)
    w_k_nope = np.asarray(inputs["w_k_nope"], np.float32)
    w_v = np.asarray(inputs["w_v"], np.float32)
    kv_g = np.asarray(inputs["kv_g"], np.float32)
    w_k_rope = np.asarray(inputs["w_k_rope"], np.float32)
    w_proj = np.asarray(inputs["w_proj"], np.float32)

    Bm = _rope_fold()
    cropeq, crope32, mask = _host_tables()
    scale = 1.0 / np.sqrt(QKD)

    wqn = w_q_nope * q_g[:, None] * scale  # [QR, H*64]
    wqr = w_q_rope * q_g[:, None] * scale  # [QR, H*32]
    wkn = w_k_nope * kv_g[:, None]  # [KVR, H*64]
    wv = w_v * kv_g[:, None]  # [KVR, H*64]
    wkr = (w_k_rope @ Bm.T) / H  # [D, 32]
    wckvkr = np.concatenate([w_ckv, wkr], axis=1)  # [D, 288]

    if "nc" not in _CACHE:
        _CACHE["nc"] = _build_nc()
    nc = _CACHE["nc"]

    in_maps = []
    for core in range(NCORES):
        b, g = divmod(core, NCORES // B)
        heads = range(HG * g, HG * (g + 1))
        wq_cols = []
        for h in heads:
            wq_cols.append(wqn[:, h * NOPE : (h + 1) * NOPE])
            wq_cols.append(wqr[:, h * ROPE : (h + 1) * ROPE] @ Bm.T)
        wq_core = np.concatenate(wq_cols, axis=1)  # [QR, 384]
        wkv_core = np.concatenate(
            [wkn[:, h * NOPE : (h + 1) * NOPE] for h in heads]
            + [wv[:, h * VD : (h + 1) * VD] for h in heads],
            axis=1,
        )  # [KVR, 512]
        wproj_core = np.concatenate(
            [w_proj[h * VD : (h + 1) * VD, :] for h in heads], axis=0
        )  # [256, D]
        in_maps.append(
            {
                "xTw": np.ascontiguousarray(x[b].T[:, W * g : W * (g + 1)]).astype(NBF),
                "cropew": np.ascontiguousarray(crope32[:, W * g : W * (g + 1)]),
                "wcq": w_cq.astype(NBF),
                "wckvkr": wckvkr.astype(NBF),
                "wq": wq_core.astype(NBF),
                "wkv": wkv_core.astype(NBF),
                "wproj": wproj_core.astype(NBF),
                "cropeq": cropeq,
                "mask": mask,
            }
        )

    res = run_bass_kernel_spmd(nc, in_maps, list(range(NCORES)))
    LAST_RESULT = res
    outs = [np.asarray(r["out"], np.float32) for r in res.results]
    gpb = NCORES // B
    out = np.stack(
        [sum(outs[b * gpb + g] for g in range(gpb)) for b in range(B)], axis=0
    )
    return out


# revision 36
# speedup vs baseline: 1.1086x; 1.0407x over previous
"""MLA (multi-head latent attention) forward on 8 TRN2 NeuronCores.

Sharding: core = 4*b + g  (b = batch 0..1, g = head-group 0..3, 4 heads each).
Each core compresses its own 512-token window, RMS-normalizes the latents in
place (rsqrt folded into the latents before the gather), AllGathers ckv+kr+cq
in ONE collective within its batch group (one op = one rendezvous + no serial
stream gaps), decompresses its 4 heads, runs causal attention over the full
2048 tokens with 128-granular triangular slicing, and projects to a bf16
partial output.  Host sums the 4 partials per batch.

Attention is organized per (head, query-half) pass: key-block (kc) outer,
query start at 128*kc, scores in [128, <=1024] PSUM chunks with a 3-deep
buffer and PV emitted two chunks behind, so the score->exp->PV chain latency
stays off the PE critical path.  The softmax denominator comes from a
ones-column in V; the PV psum is staged to SBUF so its reciprocal
(exp(-ln(d)) on ACT) and normalize run off the psum critical path.
"""

import sys

sys.path.insert(0, "/opt/trn_rl_repo")

import numpy as np
import ml_dtypes

from concourse import bacc, bass, bass_isa, mybir, tile
from concourse.bass_utils import run_bass_kernel_spmd

# problem dims (hardcoded per contract)
B, S, D = 2, 2048, 2048
H = 16
NOPE, ROPE, VD = 64, 32, 64
QR, KVR = 768, 256
EPS = 1e-6
THETA = 10000.0

HG = 4  # heads per core
NCORES = 8
P = 128
W = 512  # own-token window
NW = S // W  # 4
QKD = NOPE + ROPE  # 96
HALF = 1024  # query half (psum-chunk limit)
CKR = KVR + ROPE  # 288 latent rows for k/v
NKC = S // P  # 16 key blocks
ALLR = CKR + QR  # 1056 gathered rows

BF = mybir.dt.bfloat16
F32 = mybir.dt.float32
NBF = ml_dtypes.bfloat16
MULT = mybir.AluOpType.mult
AFT = mybir.ActivationFunctionType

LAST_RESULT = None
_CACHE = {}


def _build_nc():
    nc = bacc.Bacc("TRN2", debug=False)
    with tile.TileContext(nc) as tc:
        with (
            tc.tile_pool(name="dram", bufs=1, space="DRAM") as dram,
            tc.tile_pool(name="wres", bufs=1) as wres,
            tc.tile_pool(name="lat", bufs=1) as lat,
            tc.tile_pool(name="xin", bufs=1) as xin,
            tc.tile_pool(name="stg", bufs=2) as stg,
            tc.tile_pool(name="sqa", bufs=2) as sqa,
            tc.tile_pool(name="row", bufs=2) as rowp,
            tc.tile_pool(name="pt", bufs=4) as ptp,
            tc.tile_pool(name="rbc", bufs=2) as rbcp,
            tc.tile_pool(name="ovs", bufs=2) as ovsp,
            tc.tile_pool(name="ost", bufs=3) as ostp,
            tc.tile_pool(name="psA", bufs=3, space="PSUM") as psA,
            tc.tile_pool(name="psB", bufs=1, space="PSUM") as psB,
        ):
            # ---------------- DRAM params ----------------
            xTw = dram.tile([D, W], BF, kind="ExternalInput", name="xTw", uniquify=False)
            wcq = dram.tile([D, QR], BF, kind="ExternalInput", name="wcq", uniquify=False)
            wckvkr = dram.tile(
                [D, CKR], BF, kind="ExternalInput", name="wckvkr", uniquify=False
            )
            wq = dram.tile(
                [QR, HG * QKD], BF, kind="ExternalInput", name="wq", uniquify=False
            )
            wkv = dram.tile(
                [KVR, HG * (NOPE + VD)], BF, kind="ExternalInput", name="wkv",
                uniquify=False,
            )
            wproj = dram.tile(
                [HG * VD, D], BF, kind="ExternalInput", name="wproj", uniquify=False
            )
            cropeq_d = dram.tile(
                [QKD, S], BF, kind="ExternalInput", name="cropeq", uniquify=False
            )
            cropew_d = dram.tile(
                [ROPE, W], BF, kind="ExternalInput", name="cropew", uniquify=False
            )
            mask_d = dram.tile(
                [P, P], BF, kind="ExternalInput", name="mask", uniquify=False
            )
            out_d = dram.tile([S, D], BF, kind="ExternalOutput", name="out", uniquify=False)

            # collective buffers: one gather moves ckv+kr+cq together
            cc_in = dram.tile([ALLR, W], BF, kind="Internal", name="cc_in", uniquify=False)
            cc_o = dram.tile([NW, ALLR, W], BF, kind="Internal", name="cc_o", uniquify=False)

            GROUPS = [[0, 1, 2, 3], [4, 5, 6, 7]]

            # ---------------- resident SBUF ----------------
            # x staging shares its slot with cqT (x dies before cqT fills)
            NC_ = D // P  # 16 contraction chunks
            x_sb = xin.tile([P, NC_, W], BF, tag="big")
            wckvkr_sb = wres.tile([P, NC_, CKR], BF, tag="wckvkr")
            wcq_sb = wres.tile([P, NC_, QR], BF, tag="wcq")
            wq_sb = wres.tile([P, QR // P, HG * QKD], BF, tag="wq")
            wkv_sb = wres.tile([P, KVR // P, HG * (NOPE + VD)], BF, tag="wkv")
            wproj_sb = wres.tile([P, (HG * VD) // P, D], BF, tag="wproj")
            cropeq_sb = wres.tile([QKD, S], BF, tag="cropeq")
            cropew_sb = wres.tile([ROPE, W], BF, tag="cropew")
            mask_sb = wres.tile([P, P], BF, tag="mask")

            # x + compression weights, interleaved 2-chunk DMAs (pipelined MMs)
            for c2 in range(NC_ // 2):
                sl = slice(2 * c2 * P, (2 * c2 + 2) * P)
                nc.sync.dma_start(
                    out=x_sb[:, 2 * c2 : 2 * c2 + 2, :],
                    in_=xTw[sl, :].rearrange("(c p) w -> p c w", p=P),
                )
                nc.sync.dma_start(
                    out=wckvkr_sb[:, 2 * c2 : 2 * c2 + 2, :],
                    in_=wckvkr[sl, :].rearrange("(c p) k -> p c k", p=P),
                )
            for c2 in range(NC_ // 2):
                sl = slice(2 * c2 * P, (2 * c2 + 2) * P)
                nc.sync.dma_start(
                    out=wcq_sb[:, 2 * c2 : 2 * c2 + 2, :],
                    in_=wcq[sl, :].rearrange("(c p) q -> p c q", p=P),
                )
            # decompress/attention weights (scalar queue; needed later)
            nc.scalar.dma_start(
                out=wq_sb[:], in_=wq[:].rearrange("(c p) k -> p c k", p=P)
            )
            nc.scalar.dma_start(
                out=wkv_sb[:], in_=wkv[:].rearrange("(c p) k -> p c k", p=P)
            )
            nc.scalar.dma_start(out=cropeq_sb[:], in_=cropeq_d[:])
            nc.scalar.dma_start(out=cropew_sb[:], in_=cropew_d[:])
            nc.scalar.dma_start(out=mask_sb[:], in_=mask_d[:])
            nc.scalar.dma_start(
                out=wproj_sb[:], in_=wproj[:].rearrange("(c p) k -> p c k", p=P)
            )

            # eps row for the rsqrt (bias APs must be [P,1] SBUF)
            cb = wres.tile([P, 2], F32, tag="cb")
            nc.vector.memset(cb[:, 0:1], EPS)

            # ---------------- compression (own window) ----------------
            ckv_n = stg.tile([P, KVR // P, W], BF, tag="ckvn", bufs=1)
            cq_n = stg.tile([P, QR // P, W], BF, tag="cqn", bufs=1)
            kr_n = stg.tile([ROPE, W], BF, tag="krn", bufs=1)

            def compress_group(n_m, w_sb, raw, acc):
                for m in range(n_m):
                    ps = psA.tile([P, W], F32, tag="ps")
                    for c in range(NC_):
                        nc.tensor.matmul(
                            ps[:],
                            w_sb[:, c, m * P : (m + 1) * P],
                            x_sb[:, c, :],
                            start=(c == 0),
                            stop=(c == NC_ - 1),
                        )
                    nc.vector.tensor_copy(out=raw[:, m, :], in_=ps[:])
                    sq = sqa.tile([P, W], BF, tag="sq")
                    nc.scalar.square(out=sq[:], in_=ps[:])
                    if m == 0:
                        nc.vector.tensor_copy(out=acc[:], in_=sq[:])
                    else:
                        nc.vector.tensor_add(out=acc[:], in0=acc[:], in1=sq[:])

            def rsqrt_bcast(acc, inv_n):
                red = sqa.tile([P, W], F32, tag="red")
                nc.gpsimd.partition_all_reduce(
                    red[:], acc[:], channels=P, reduce_op=bass_isa.ReduceOp.add
                )
                srow = rowp.tile([1, W], F32, tag="srow")
                nc.scalar.activation(
                    out=srow[:], in_=red[0:1, :], func=AFT.Sqrt,
                    bias=cb[0:1, 0:1], scale=inv_n,
                )
                rrow = rowp.tile([1, W], F32, tag="rrow")
                nc.vector.reciprocal(out=rrow[:], in_=srow[:])
                rbc = sqa.tile([P, W], F32, tag="rbc")
                nc.gpsimd.partition_broadcast(rbc[:], rrow[:])
                return rbc

            # --- ckv (2 m-tiles) + kr ---
            acc_kv = sqa.tile([P, W], F32, tag="acckv", bufs=1)
            compress_group(KVR // P, wckvkr_sb, ckv_n, acc_kv)
            pkr = psA.tile([ROPE, W], F32, tag="ps")
            for c in range(NC_):
                nc.tensor.matmul(
                    pkr[:],
                    wckvkr_sb[:, c, KVR:CKR],
                    x_sb[:, c, :],
                    start=(c == 0),
                    stop=(c == NC_ - 1),
                )
            nc.vector.tensor_tensor(out=kr_n[:], in0=pkr[:], in1=cropew_sb[:], op=MULT)
            nc.sync.dma_start(out=cc_in[KVR:CKR, :], in_=kr_n[:])
            rbc_kv = rsqrt_bcast(acc_kv, 1.0 / KVR)
            for m in range(KVR // P):
                nc.vector.tensor_tensor(
                    out=ckv_n[:, m, :], in0=ckv_n[:, m, :], in1=rbc_kv[:], op=MULT
                )
            nc.sync.dma_start(
                out=cc_in[0:KVR, :].rearrange("(m p) w -> p m w", p=P), in_=ckv_n[:]
            )

            # --- cq (6 m-tiles) ---
            acc_q = sqa.tile([P, W], F32, tag="accq", bufs=1)
            compress_group(QR // P, wcq_sb, cq_n, acc_q)
            rbc_q = rsqrt_bcast(acc_q, 1.0 / QR)
            for m in range(QR // P):
                nc.vector.tensor_tensor(
                    out=cq_n[:, m, :], in0=cq_n[:, m, :], in1=rbc_q[:], op=MULT
                )
            nc.sync.dma_start(
                out=cc_in[CKR:ALLR, :].rearrange("(m p) w -> p m w", p=P), in_=cq_n[:]
            )

            # --- the one collective ---
            nc.gpsimd.collective_compute(
                "AllGather",
                mybir.AluOpType.bypass,
                replica_groups=GROUPS,
                ins=[cc_in[:]],
                outs=[cc_o[:]],
            )

            # ---------------- latent tiles (full sequence) ----------------
            ckvT = lat.tile([P, KVR // P, S], BF, tag="ckvT")
            krT = lat.tile([ROPE, S], BF, tag="krT")
            cqT = xin.tile([P, QR // P, S], BF, tag="big")  # reuses x slot

            for w in range(NW):
                ws = slice(w * W, (w + 1) * W)
                nc.sync.dma_start(
                    out=cqT[:, :, ws],
                    in_=cc_o[w, CKR:ALLR, :].rearrange("(m p) w -> p m w", p=P),
                )
                nc.scalar.dma_start(
                    out=ckvT[:, :, ws],
                    in_=cc_o[w, 0:KVR, :].rearrange("(m p) w -> p m w", p=P),
                )
                nc.scalar.dma_start(out=krT[:, ws], in_=cc_o[w, KVR:CKR, :])

            # ---------------- decompress ----------------
            kT = [lat.tile([QKD, S], BF, tag=f"kT{h}", name=f"kT{h}") for h in range(HG)]
            qT = [lat.tile([QKD, S], BF, tag=f"qT{h}", name=f"qT{h}") for h in range(HG)]
            vaug = lat.tile([P, NKC, HG, VD + 1], BF, tag="vaug")
            oT = lat.tile([P, 2, S], BF, tag="oT")
            nc.vector.memset(vaug[:, :, :, VD : VD + 1], 1.0)

            # q per (head, half): psum [96, HALF]; rope rows get cos at evict
            for h in range(HG):
                for q2 in range(S // HALF):
                    hs = slice(q2 * HALF, (q2 + 1) * HALF)
                    ps = psA.tile([QKD, HALF], F32, tag="ps")
                    for r in range(QR // P):
                        for s2 in range(2):
                            nc.tensor.matmul(
                                ps[:, s2 * W : (s2 + 1) * W],
                                wq_sb[:, r, h * QKD : (h + 1) * QKD],
                                cqT[:, r, q2 * HALF + s2 * W : q2 * HALF + (s2 + 1) * W],
                                start=(r == 0),
                                stop=(r == QR // P - 1),
                            )
                    nc.vector.tensor_tensor(
                        out=qT[h][:, hs], in0=ps[:], in1=cropeq_sb[:, hs], op=MULT
                    )

            # k_nope per (head, half): psum [64, HALF]
            for h in range(HG):
                for q2 in range(S // HALF):
                    hs = slice(q2 * HALF, (q2 + 1) * HALF)
                    ps = psA.tile([NOPE, HALF], F32, tag="ps")
                    for r in range(KVR // P):
                        for s2 in range(2):
                            nc.tensor.matmul(
                                ps[:, s2 * W : (s2 + 1) * W],
                                wkv_sb[:, r, h * NOPE : (h + 1) * NOPE],
                                ckvT[:, r, q2 * HALF + s2 * W : q2 * HALF + (s2 + 1) * W],
                                start=(r == 0),
                                stop=(r == KVR // P - 1),
                            )
                    nc.vector.tensor_copy(out=kT[h][0:NOPE, hs], in_=ps[:])
                nc.vector.tensor_copy(out=kT[h][NOPE:QKD, :], in_=krT[:])

            # v token-major: psum [128 tokens, HG*VD]
            for ck in range(NKC):
                ps = psA.tile([P, HG * VD], F32, tag="ps")
                for r in range(KVR // P):
                    nc.tensor.matmul(
                        ps[:],
                        ckvT[:, r, ck * P : (ck + 1) * P],
                        wkv_sb[:, r, HG * NOPE : HG * (NOPE + VD)],
                        start=(r == 0),
                        stop=(r == KVR // P - 1),
                    )
                nc.vector.tensor_copy(
                    out=vaug[:, ck, :, 0:VD],
                    in_=ps[:].rearrange("p (h d) -> p h d", h=HG),
                )

            # ---------------- attention ----------------
            # per (head, query-half) pass: kc outer, query start at 128*kc.
            # single opv psum per pass (psB); scores triple-buffered (psA);
            # PV lags two chunks so the score->exp->(mask)->PV chain latency
            # hides under pipelined PE work.
            def pieces(q0, q1):
                out = []
                a = q0
                while a < q1:
                    b = min((a // W + 1) * W, q1)
                    out.append((a, b))
                    a = b
                return out

            for h in range(HG):
                ostage = []
                for qh in range(2):
                    opv = psB.tile(
                        [VD + 1, HALF], F32, tag="opv", name=f"opv{h}_{qh}"
                    )
                    last_kc = (qh + 1) * (NKC // 2) - 1
                    pending = []

                    def flush_pv(n, opv=opv, qh=qh, h=h, last_kc=last_kc, pending=pending):
                        while len(pending) > n:
                            kc_, q0_, q1_, pt_, masked = pending.pop(0)
                            pcs = pieces(q0_, q1_)
                            if masked:  # emit non-masked pieces first
                                pcs = pcs[1:] + pcs[:1]
                            for a, b in pcs:
                                nc.tensor.matmul(
                                    opv[:, a - qh * HALF : b - qh * HALF],
                                    vaug[:, kc_, h, :],
                                    pt_[:, a - q0_ : b - q0_],
                                    start=(kc_ == 0),
                                    stop=(kc_ == last_kc),
                                )

                    for kc in range(last_kc + 1):
                        q0 = max(P * kc, qh * HALF)
                        q1 = (qh + 1) * HALF
                        sp = psA.tile([P, HALF], F32, tag="ps")
                        for a, b in pieces(q0, q1):
                            nc.tensor.matmul(
                                sp[:, a - qh * HALF : b - qh * HALF],
                                kT[h][:, kc * P : (kc + 1) * P],
                                qT[h][:, a:b],
                                start=True,
                                stop=True,
                            )
                        pt = ptp.tile([P, HALF], BF, tag="pt")
                        nc.scalar.activation(
                            out=pt[:, 0 : q1 - q0],
                            in_=sp[:, q0 - qh * HALF : HALF],
                            func=AFT.Exp,
                        )
                        masked = q0 == P * kc
                        if masked:  # diagonal chunk: mask first 128 cols
                            nc.vector.tensor_tensor(
                                out=pt[:, 0:P], in0=pt[:, 0:P], in1=mask_sb[:], op=MULT
                            )
                        pending.append((kc, q0, q1, pt, masked))
                        flush_pv(2)
                    flush_pv(0)

                    # stage the pass psum out so the denominator/normalize
                    # chain runs off the psum critical path
                    ov = ovsp.tile([VD + 1, HALF], BF, tag="ovs")
                    nc.vector.tensor_copy(out=ov[:], in_=opv[:])
                    ostage.append(ov)

                # normalize: rec = exp(-ln(denom)), broadcast, multiply.
                # both Lns then both Exps: two ACT table loads per head
                lnrs = []
                for qh in range(2):
                    lnr = rowp.tile([1, HALF], F32, tag="lnr", name=f"lnr{h}_{qh}")
                    nc.scalar.activation(
                        out=lnr[:], in_=ostage[qh][VD : VD + 1, :], func=AFT.Ln
                    )
                    lnrs.append(lnr)
                for qh in range(2):
                    rec = rowp.tile([1, HALF], BF, tag="rec", name=f"rec{h}_{qh}")
                    nc.scalar.activation(
                        out=rec[:], in_=lnrs[qh][:], func=AFT.Exp, scale=-1.0
                    )
                    rbc = rbcp.tile([VD, HALF], BF, tag="rbc")
                    nc.gpsimd.partition_broadcast(rbc[:], rec[:])
                    oTh = oT[VD * (h % 2) : VD * (h % 2) + VD, h // 2, :]
                    nc.vector.tensor_tensor(
                        out=oTh[:, qh * HALF : (qh + 1) * HALF],
                        in0=ostage[qh][0:VD, :],
                        in1=rbc[:],
                        op=MULT,
                    )

            # ---------------- projection ----------------
            for t in range(S // P):
                pps = []
                for wc in range(2):
                    pp = psA.tile([P, HALF], F32, tag="ps", name=f"pp{t}_{wc}")
                    for i in range(2):
                        for s2 in range(2):
                            nc.tensor.matmul(
                                pp[:, s2 * W : (s2 + 1) * W],
                                oT[:, i, t * P : (t + 1) * P],
                                wproj_sb[:, i, wc * HALF + s2 * W : wc * HALF + (s2 + 1) * W],
                                start=(i == 0),
                                stop=(i == 1),
                            )
                    pps.append(pp)
                for wc, pp in enumerate(pps):
                    o = ostp.tile([P, HALF], BF, tag="ost")
                    if wc == 0:
                        nc.vector.tensor_copy(out=o[:], in_=pp[:])
                    else:
                        nc.scalar.copy(out=o[:], in_=pp[:])
                    nc.sync.dma_start(
                        out=out_d[t * P : (t + 1) * P, wc * HALF : (wc + 1) * HALF],
                        in_=o[:],
                    )

    nc.compile()
    return nc


def _rope_fold():
    """32x32 butterfly for RoPE with the reference's sin==cos bug."""
    Bm = np.zeros((ROPE, ROPE), np.float32)
    for j in range(ROPE // 2):
        Bm[2 * j, 2 * j] = 1.0
        Bm[2 * j, 2 * j + 1] = -1.0
        Bm[2 * j + 1, 2 * j] = 1.0
        Bm[2 * j + 1, 2 * j + 1] = 1.0
    return Bm


def _host_tables():
    freqs = 1.0 / (THETA ** (np.arange(0, ROPE, 2, dtype=np.float32) / ROPE))
    ang = np.outer(np.arange(S, dtype=np.float32), freqs)  # [S, 16]
    cos = np.cos(ang)
    crope32 = np.repeat(cos, 2, axis=1).T.copy().astype(np.float32)  # [32, S]
    cropeq = np.concatenate([np.ones((NOPE, S), np.float32), crope32], axis=0)
    mask = np.zeros((P, P), np.float32)
    for k in range(P):
        mask[k, k:] = 1.0
    return cropeq.astype(NBF), crope32.astype(NBF), mask.astype(NBF)


def kernel(**inputs):
    global LAST_RESULT
    x = np.asarray(inputs["x"], np.float32)
    w_cq = np.asarray(inputs["w_cq"], np.float32)
    w_q_nope = np.asarray(inputs["w_q_nope"], np.float32)
    w_q_rope = np.asarray(inputs["w_q_rope"], np.float32)
    q_g = np.asarray(inputs["q_g"], np.float32)
    w_ckv = np.asarray(inputs["w_ckv"], np.float32)
    w_k_nope = np.asarray(inputs["w_k_nope"], np.float32)
    w_v = np.asarray(inputs["w_v"], np.float32)
    kv_g = np.asarray(inputs["kv_g"], np.float32)
    w_k_rope = np.asarray(inputs["w_k_rope"], np.float32)
    w_proj = np.asarray(inputs["w_proj"], np.float32)

    Bm = _rope_fold()
    cropeq, crope32, mask = _host_tables()
    scale = 1.0 / np.sqrt(QKD)

    wqn = w_q_nope * q_g[:, None] * scale  # [QR, H*64]
    wqr = w_q_rope * q_g[:, None] * scale  # [QR, H*32]
    wkn = w_k_nope * kv_g[:, None]  # [KVR, H*64]
    wv = w_v * kv_g[:, None]  # [KVR, H*64]
    wkr = (w_k_rope @ Bm.T) / H  # [D, 32]
    wckvkr = np.concatenate([w_ckv, wkr], axis=1)  # [D, 288]

    if "nc" not in _CACHE:
        _CACHE["nc"] = _build_nc()
    nc = _CACHE["nc"]

    in_maps = []
    for core in range(NCORES):
        b, g = divmod(core, NCORES // B)
        heads = range(HG * g, HG * (g + 1))
        wq_cols = []
        for h in heads:
            wq_cols.append(wqn[:, h * NOPE : (h + 1) * NOPE])
            wq_cols.append(wqr[:, h * ROPE : (h + 1) * ROPE] @ Bm.T)
        wq_core = np.concatenate(wq_cols, axis=1)  # [QR, 384]
        wkv_core = np.concatenate(
            [wkn[:, h * NOPE : (h + 1) * NOPE] for h in heads]
            + [wv[:, h * VD : (h + 1) * VD] for h in heads],
            axis=1,
        )  # [KVR, 512]
        wproj_core = np.concatenate(
            [w_proj[h * VD : (h + 1) * VD, :] for h in heads], axis=0
        )  # [256, D]
        in_maps.append(
            {
                "xTw": np.ascontiguousarray(x[b].T[:, W * g : W * (g + 1)]).astype(NBF),
                "cropew": np.ascontiguousarray(crope32[:, W * g : W * (g + 1)]).astype(NBF),
                "wcq": w_cq.astype(NBF),
                "wckvkr": wckvkr.astype(NBF),
                "wq": wq_core.astype(NBF),
                "wkv": wkv_core.astype(NBF),
                "wproj": wproj_core.astype(NBF),
                "cropeq": cropeq,
                "mask": mask,
            }
        )

    res = run_bass_kernel_spmd(nc, in_maps, list(range(NCORES)))
    LAST_RESULT = res
    outs = [np.asarray(r["out"], np.float32) for r in res.results]
    gpb = NCORES // B
    out = np.stack(
        [sum(outs[b * gpb + g] for g in range(gpb)) for b in range(B)], axis=0
    )
    return out
